# revision 2
# baseline (speedup 1.0000x reference)
"""Causal single-head attention (B=4, T=4096, C=1024, D=64) on 8 NeuronCores.

Sharding: core c = (batch b = c % 4, half h = c // 4).
Each core handles ALL queries of its batch against its half of the key
blocks (256-token blocks with block index ≡ h mod 2).  Pure SPMD: the
program is identical on every core; cores differ only in input data
(x[b]^T, block-pair-permuted for h=1, and the causal mask).  Each core
emits unnormalized U[q, 0:64] = sum_k exp(s) v and U[q, 64] = sum_k
exp(s); the host adds the two halves per batch and normalizes.

On-chip dataflow (bf16 compute, f32 PSUM accumulation):
  per 128-token tile tt:  PJ [128t, 192] = sum_c x_tile[c,tt]^T @ w[c]
    (fused Q|K|V projection, x tile stationary; non-key tiles: Q only)
  Q^T, K^T via PE transposes (deferred one unit for pipelining);
  V kept in [token, d] layout + ones column (softmax denominator)
  per query supertile st (512 q), local key tile pairs jp (diag first):
    S^T [128k, 2x512q] = K^T[j] @ Q^T[st]     (contraction over D=64)
    P^T = exp(S^T / 8)                        (one ACT instr per pair)
    diagonal pair: P^T *= mask                (DVE)
    U [128q, 65] += P^T-chunk^T @ [V_j | 1]   (P chunk stationary,
        deferred TWO pairs so the PE stays off the exp critical path)
  Projection units of the NEXT supertile interleave with attention
  pairs as fillers.  U -> SBUF -> DMA out per supertile (SWDGE/Pool
  path, keeping HWDGE free for input streaming).
"""
import sys
import numpy as np
import ml_dtypes
from collections import deque

if "/opt/trn_rl_repo" not in sys.path:
    sys.path.insert(0, "/opt/trn_rl_repo")

import concourse.bacc as bacc
import concourse.mybir as mybir
from concourse import tile
from concourse import bass_utils

bf16 = mybir.dt.bfloat16
f32 = mybir.dt.float32
BF = ml_dtypes.bfloat16

B, T, C, D = 4, 4096, 1024, 64
NC_ = C // 128      # 8 c-tiles
NTT = T // 128      # 32 token tiles
NST = 8             # query supertiles (512 q each)
STQ = 512

_CACHE = {}


def _build():
    nc = bacc.Bacc(None, target_bir_lowering=False, debug=False, num_devices=8)

    # xq tile-packed: xq[:, 1024*tt + 128*c : +128] = x^T[c-tile, token tile tt]
    xq = nc.dram_tensor("xq", [128, NTT * 1024], bf16, kind="ExternalInput")
    w = nc.dram_tensor("w", [128, NC_ * 192], bf16, kind="ExternalInput")
    # aux2 = diag mask [0:1024] | identity [1024:1152]
    aux2 = nc.dram_tensor("aux2", [128, 1152], bf16, kind="ExternalInput")
    out = nc.dram_tensor("out", [128, NST * 260], f32, kind="ExternalOutput")

    with tile.TileContext(nc) as tc:
        with tc.tile_pool(name="sb", bufs=1) as sb, \
             tc.tile_pool(name="qk", bufs=3) as qkp, \
             tc.tile_pool(name="pp", bufs=3) as pp, \
             tc.tile_pool(name="uo", bufs=2) as uop, \
             tc.tile_pool(name="ps", bufs=2, space="PSUM") as ps:

            # ---- resident tiles ----
            xq_sb = sb.tile([128, NTT * 1024], bf16, tag="xq")
            w_sb = sb.tile([128, NC_ * 192], bf16, tag="w")
            aux_sb = sb.tile([128, 1152], bf16, tag="aux")
            msk_sb = aux_sb[:, 0:1024]
            idn_sb = aux_sb[:, 1024:1152]
            qT = sb.tile([64, T], bf16, tag="qT")       # Q^T strip
            kT = sb.tile([64, T // 2], bf16, tag="kT")  # K^T local tiles
            v_sb = sb.tile([128, 16 * 65], bf16, tag="v")  # [V_j | 1] tiles

            # DMA order = consumption order: w, then one DMA per token tile
            # (each delivers ALL c-tiles of that tile, so projection unit tt
            # unblocks as soon as ITS chunk lands).
            nc.sync.dma_start(w_sb[:], w[:])
            nc.sync.dma_start(aux_sb[:], aux2[:])  # identity gates transposes
            for tt in range(NTT):
                nc.sync.dma_start(xq_sb[:, 1024 * tt:1024 * (tt + 1)],
                                  xq[:, 1024 * tt:1024 * (tt + 1)])
            # ones columns of the V tiles (denominator trick)
            for j in range(16):
                nc.gpsimd.memset(v_sb[:, 65 * j + 64:65 * j + 65], 1.0)

            # PE p-state warmup: the PE ramps to full clock only after ~3us
            # of continuous busy; run junk matmuls on a zeroed tile right at
            # t=0 so the ramp completes before the first real projection.
            warm = sb.tile([128, 640], bf16, tag="warm")
            nc.vector.memset(warm[:], 0.0)
            for i in range(5):
                wps = ps.tile([128, 512], f32, tag="s", name=f"warm{i}")
                nc.tensor.matmul(wps[:], warm[:, 0:128], warm[:, 128:640],
                                 start=True, stop=True)

            # ---- projection units, software-pipelined ----
            # mm-phase: fused projection matmuls into PSUM + DVE copies out.
            # tp-phase (transposes + Q^T/K^T copies) is deferred one unit so
            # the PE never waits on the just-issued DVE copy.
            proj_prev = [None]

            def xsrc(tt, c):
                return xq_sb[:, 1024 * tt + 128 * c:1024 * tt + 128 * (c + 1)]

            def proj_tp(state):
                tt, qk, tp = state
                is_key = ((tt >> 1) & 1) == 0
                if is_key:
                    j = 2 * (tt // 4) + (tt & 1)
                    nc.tensor.transpose(tp[:, 0:128], qk[:, 0:64], idn_sb)
                    nc.tensor.transpose(tp[:, 128:256], qk[:, 64:128], idn_sb)
                    nc.vector.tensor_copy(qT[:, 128 * tt:128 * (tt + 1)],
                                          tp[:, 0:128])
                    nc.vector.tensor_copy(kT[:, 128 * j:128 * (j + 1)],
                                          tp[:, 128:256])
                else:
                    nc.tensor.transpose(tp[:, 0:128], qk[:], idn_sb)
                    nc.vector.tensor_copy(qT[:, 128 * tt:128 * (tt + 1)],
                                          tp[:, 0:128])

            def proj_unit(tt):
                """Key-ness fixed to (tt//2)%2==0; h=1 cores get block-pair-
                permuted inputs so their key blocks land on even blocks."""
                is_key = ((tt >> 1) & 1) == 0
                wid = 192 if is_key else 64
                pj = ps.tile([128, wid], f32, tag="pj", name=f"pj{tt}")
                for c in range(NC_):
                    nc.tensor.matmul(pj[:], xsrc(tt, c),
                                     w_sb[:, 192 * c:192 * c + wid],
                                     start=(c == 0), stop=(c == NC_ - 1))
                qk = qkp.tile([128, 128 if is_key else 64], bf16, tag="qk",
                              name=f"qk{tt}")
                if is_key:
                    j = 2 * (tt // 4) + (tt & 1)
                    nc.vector.tensor_copy(qk[:], pj[:, 0:128])
                    nc.vector.tensor_copy(v_sb[:, 65 * j:65 * j + 64],
                                          pj[:, 128:192])
                else:
                    nc.vector.tensor_copy(qk[:], pj[:])
                tp = ps.tile([64, 256], bf16, tag="pj", name=f"tp{tt}")
                if proj_prev[0] is not None:
                    proj_tp(proj_prev[0])
                proj_prev[0] = (tt, qk, tp)

            def proj_flush():
                if proj_prev[0] is not None:
                    proj_tp(proj_prev[0])
                    proj_prev[0] = None

            # ---- attention: one continuous pair stream across supertiles ----
            # Global software pipeline: the U matmuls of a pair are deferred
            # two pairs (possibly crossing into the next supertile) so the
            # PE never sits on the ACT exp critical path, and the ACT stream
            # has no supertile-boundary bubble.
            u_state = {}   # st -> (u4 tile, n_emitted)
            pendings = []  # (st, jp, p2)

            def emit_u(st, jp, p2):
                if st not in u_state:
                    u_state[st] = [ps.tile([128, 260], f32, tag="u",
                                           name=f"u{st}", bufs=1), 0]
                ent = u_state[st]
                u4 = ent[0]
                for d in range(2):
                    j = 2 * jp + d
                    for g in range(4):
                        # start=True zeroes the WHOLE PSUM bank: set it only
                        # on the chronologically first matmul into u4.
                        nc.tensor.matmul(
                            u4[:, 65 * g:65 * (g + 1)],
                            p2[:, 512 * d + 128 * g:512 * d + 128 * (g + 1)],
                            v_sb[:, 65 * j:65 * (j + 1)],
                            start=(ent[1] == 0),
                            stop=(ent[1] == 8 * (st + 1) - 1),
                            skip_group_check=True)
                        ent[1] += 1
                if ent[1] == 8 * (st + 1):  # supertile complete -> ship it
                    uo_t = uop.tile([128, 260], f32, tag="uo", name=f"uo{st}")
                    nc.vector.tensor_copy(uo_t[:], u4[:])
                    eng = nc.sync if st == NST - 1 else nc.gpsimd
                    eng.dma_start(out[:, 260 * st:260 * (st + 1)], uo_t[:])
                    del u_state[st]

            def emit_pair(st, jp):
                qsl = slice(STQ * st, STQ * (st + 1))
                s2 = ps.tile([128, 1024], f32, tag="s", name=f"s{st}_{jp}")
                p2 = pp.tile([128, 1024], bf16, tag="p", name=f"p{st}_{jp}")
                for d in range(2):
                    j = 2 * jp + d
                    nc.tensor.matmul(s2[:, 512 * d:512 * (d + 1)],
                                     kT[:, 128 * j:128 * (j + 1)],
                                     qT[:, qsl], start=True, stop=True)
                nc.scalar.activation(p2[:], s2[:],
                                     mybir.ActivationFunctionType.Exp,
                                     scale=0.125)
                if jp == st:  # diagonal pair -> causal mask
                    nc.vector.tensor_mul(p2[:], p2[:], msk_sb)
                pendings.append((st, jp, p2))
                if len(pendings) > 2:
                    emit_u(*pendings.pop(0))

            def filler_q(qtr, half):
                return deque(
                    (lambda t: (lambda: proj_unit(t)))(8 * qtr + 4 * half + i)
                    for i in range(4))

            # Fillers (projection units of supertile st+1) are emitted inside
            # supertile st's pair loop, starting at the pair index where their
            # DMA chunk has arrived (the stream is DMA-paced early on).
            for tl in range(4):
                proj_unit(tl)
            schedule = [filler_q(0, 1), filler_q(1, 0), filler_q(1, 1),
                        filler_q(2, 0), filler_q(2, 1), filler_q(3, 0),
                        filler_q(3, 1), deque()]
            for st in range(8):
                fillers = schedule[st]
                proj_flush()  # this supertile's Q^T/K^T must be complete
                f0 = 3 if st <= 4 else 0
                for pi, jp in enumerate([st] + list(range(st))):  # diag first
                    emit_pair(st, jp)
                    if fillers and pi >= f0:
                        fillers.popleft()()
                while fillers:
                    fillers.popleft()()
            for pd in pendings:
                emit_u(*pd)
            pendings.clear()

    nc.compile()
    return nc


def _get_nc():
    if "nc" not in _CACHE:
        _CACHE["nc"] = _build()
    return _CACHE["nc"]


def kernel(x, Wq, Wk, Wv, _trace=False):
    x = np.asarray(x)
    nc = _get_nc()

    # Token permutation per half: the program treats 256-token blocks with
    # block index ≡ 0 (mod 2) as key blocks.  For h=1 cores we swap each
    # even/odd 256-block pair so THEIR key blocks land on even positions.
    tok = np.arange(T)
    blk = tok // 256
    perm1 = np.where(blk % 2 == 0, tok + 256, tok - 256)  # swap block pairs

    xT = np.ascontiguousarray(x.transpose(0, 2, 1)).astype(BF)   # [B, C, T]
    xT1 = np.ascontiguousarray(xT[:, :, perm1])

    w_all = np.concatenate([Wq, Wk, Wv], axis=1).astype(np.float32)  # [C, 192]
    w_packed = np.ascontiguousarray(
        w_all.reshape(NC_, 128, 192).transpose(1, 0, 2).reshape(128, NC_ * 192)
    ).astype(BF)
    idn = np.eye(128, dtype=BF)

    # Masks for the diagonal pair (local key tiles (2st, 2st+1)); the causal
    # test runs on GLOBAL token ids, mapped through the per-half permutation.
    i = np.arange(STQ)[None, :]     # query offset within supertile (permuted)
    k = np.arange(128)[:, None]     # key offset within tile (permuted)
    masks = {}
    for h in range(2):
        if h == 0:
            gq = i
            gk = k
        else:
            gq = np.where(i < 256, i + 256, i - 256)
            gk = k + 256
        m = np.concatenate([(gk <= gq), (gk + 128 <= gq)], axis=0)  # [256,512]
        masks[h] = np.concatenate([m[0:128], m[128:256]],
                                  axis=1).astype(BF)  # [128, 1024]

    def pack_tiles(xTb):
        # [C, T] -> [128, tt*1024 + c*128 + t]
        return np.ascontiguousarray(
            xTb.reshape(NC_, 128, NTT, 128).transpose(1, 2, 0, 3)
            .reshape(128, NTT * 1024))

    in_maps = []
    for c in range(8):
        b, h = c % 4, c // 4
        xTb = xT[b] if h == 0 else xT1[b]
        in_maps.append({
            "xq": pack_tiles(xTb),
            "w": w_packed,
            "aux2": np.concatenate([masks[h], idn], axis=1),
        })

    res = bass_utils.run_bass_kernel_spmd(nc, in_maps, core_ids=list(range(8)),
                                          trace=_trace)
    _CACHE["last_results"] = res

    # Decode: U[c] [128, 8*260] -> [q_perm, 65]; un-permute h=1 tokens.
    O = np.empty((B, T, D), dtype=np.float32)
    for b in range(B):
        Uh = []
        for h in range(2):
            U = res.results[b + 4 * h]["out"]            # [128, 2080]
            U = U.reshape(128, NST, 4, 65).transpose(1, 2, 0, 3)
            U = U.reshape(T, 65)                          # permuted q order
            Uh.append(U[perm1] if h == 1 else U)          # global q order
        Ut = Uh[0] + Uh[1]
        O[b] = Ut[:, 0:64] / Ut[:, 64:65]
    return O


# revision 3
# speedup vs baseline: 1.0309x; 1.0309x over previous
"""Causal single-head attention (B=4, T=4096, C=1024, D=64) on 8 NeuronCores.

Sharding: core c = (batch b = c % 4, half h = c // 4).
Each core handles ALL queries of its batch against its half of the key
blocks (256-token blocks with block index ≡ h mod 2).  Pure SPMD: the
program is identical on every core; cores differ only in input data
(x[b]^T, block-pair-permuted for h=1, and the causal mask).  Each core
emits unnormalized U[q, 0:64] = sum_k exp(s) v and U[q, 64] = sum_k
exp(s); the host adds the two halves per batch and normalizes.

On-chip dataflow (bf16 compute, f32 PSUM accumulation):
  per 128-token tile tt:  PJ [128t, 192] = sum_c x_tile[c,tt]^T @ w[c]
    (fused Q|K|V projection, x tile stationary; non-key tiles: Q only)
  Q^T, K^T via PE transposes (deferred one unit for pipelining);
  V kept in [token, d] layout + ones column (softmax denominator)
  per query supertile st (512 q), local key tile pairs jp (diag first):
    S^T [128k, 2x512q] = K^T[j] @ Q^T[st]     (contraction over D=64)
    P^T = exp(S^T / 8)                        (one ACT instr per pair)
    diagonal pair: P^T *= mask                (DVE)
    U [128q, 65] += P^T-chunk^T @ [V_j | 1]   (P chunk stationary,
        deferred TWO pairs so the PE stays off the exp critical path)
  Projection units of the NEXT supertile interleave with attention
  pairs as fillers.  U -> SBUF -> DMA out per supertile (SWDGE/Pool
  path, keeping HWDGE free for input streaming).
"""
import sys
import numpy as np
import ml_dtypes
from collections import deque

if "/opt/trn_rl_repo" not in sys.path:
    sys.path.insert(0, "/opt/trn_rl_repo")

import concourse.bacc as bacc
import concourse.mybir as mybir
from concourse import tile
from concourse import bass_utils

bf16 = mybir.dt.bfloat16
f32 = mybir.dt.float32
BF = ml_dtypes.bfloat16

B, T, C, D = 4, 4096, 1024, 64
NC_ = C // 128      # 8 c-tiles
NTT = T // 128      # 32 token tiles
NST = 8             # query supertiles (512 q each)
STQ = 512

_CACHE = {}


def _build():
    nc = bacc.Bacc(None, target_bir_lowering=False, debug=False, num_devices=8)

    # xq tile-packed: xq[:, 1024*tt + 128*c : +128] = x^T[c-tile, token tile tt]
    xq = nc.dram_tensor("xq", [128, NTT * 1024], bf16, kind="ExternalInput")
    w = nc.dram_tensor("w", [128, NC_ * 192], bf16, kind="ExternalInput")
    # aux2 = diag mask [0:1024] | identity [1024:1152]
    aux2 = nc.dram_tensor("aux2", [128, 1152], bf16, kind="ExternalInput")
    out = nc.dram_tensor("out", [128, NST * 260], f32, kind="ExternalOutput")

    with tile.TileContext(nc) as tc:
        with tc.tile_pool(name="sb", bufs=1) as sb, \
             tc.tile_pool(name="qk", bufs=3) as qkp, \
             tc.tile_pool(name="pp", bufs=4) as pp, \
             tc.tile_pool(name="uo", bufs=2) as uop, \
             tc.tile_pool(name="ps", bufs=2, space="PSUM") as ps:

            # ---- resident tiles ----
            xq_sb = sb.tile([128, NTT * 1024], bf16, tag="xq")
            w_sb = sb.tile([128, NC_ * 192], bf16, tag="w")
            aux_sb = sb.tile([128, 1152], bf16, tag="aux")
            msk_sb = aux_sb[:, 0:1024]
            idn_sb = aux_sb[:, 1024:1152]
            qT = sb.tile([64, T], bf16, tag="qT")       # Q^T strip
            kT = sb.tile([64, T // 2], bf16, tag="kT")  # K^T local tiles
            v_sb = sb.tile([128, 16 * 65], bf16, tag="v")  # [V_j | 1] tiles

            # DMA order = consumption order: w, then one DMA per token tile
            # (each delivers ALL c-tiles of that tile, so projection unit tt
            # unblocks as soon as ITS chunk lands).
            nc.sync.dma_start(w_sb[:], w[:])
            for tt in range(NTT):
                nc.sync.dma_start(xq_sb[:, 1024 * tt:1024 * (tt + 1)],
                                  xq[:, 1024 * tt:1024 * (tt + 1)])
                if tt == 2:  # identity gates the first transposes (~chunk3)
                    nc.sync.dma_start(aux_sb[:], aux2[:])
            # ones columns of the V tiles (denominator trick)
            for j in range(16):
                nc.gpsimd.memset(v_sb[:, 65 * j + 64:65 * j + 65], 1.0)

            # PE p-state warmup: the PE ramps to full clock only after ~3us
            # of continuous busy; run junk matmuls on a zeroed tile right at
            # t=0 so the ramp completes before the first real projection.
            warm = sb.tile([128, 640], bf16, tag="warm")
            nc.vector.memset(warm[:], 0.0)
            for i in range(5):
                wps = ps.tile([128, 512], f32, tag="s", name=f"warm{i}")
                nc.tensor.matmul(wps[:], warm[:, 0:128], warm[:, 128:640],
                                 start=True, stop=True)

            # ---- projection units, software-pipelined ----
            # mm-phase: fused projection matmuls into PSUM + DVE copies out.
            # tp-phase (transposes + Q^T/K^T copies) is deferred one unit so
            # the PE never waits on the just-issued DVE copy.
            proj_prev = [None]

            def xsrc(tt, c):
                return xq_sb[:, 1024 * tt + 128 * c:1024 * tt + 128 * (c + 1)]

            def proj_tp(state):
                tt, qk, tp = state
                is_key = ((tt >> 1) & 1) == 0
                if is_key:
                    j = 2 * (tt // 4) + (tt & 1)
                    nc.tensor.transpose(tp[:, 0:128], qk[:, 0:64], idn_sb)
                    nc.tensor.transpose(tp[:, 128:256], qk[:, 64:128], idn_sb)
                    nc.vector.tensor_copy(qT[:, 128 * tt:128 * (tt + 1)],
                                          tp[:, 0:128])
                    nc.vector.tensor_copy(kT[:, 128 * j:128 * (j + 1)],
                                          tp[:, 128:256])
                else:
                    nc.tensor.transpose(tp[:, 0:128], qk[:], idn_sb)
                    nc.vector.tensor_copy(qT[:, 128 * tt:128 * (tt + 1)],
                                          tp[:, 0:128])

            def proj_unit(tt):
                """Key-ness fixed to (tt//2)%2==0; h=1 cores get block-pair-
                permuted inputs so their key blocks land on even blocks."""
                is_key = ((tt >> 1) & 1) == 0
                if not is_key and tt % 4 == 3 and tt <= 19:
                    # The tile that gates attention(st): compute Q^T directly
                    # (w stationary), skipping the qk-copy + transpose hops,
                    # with the qT copy on ACT to dodge the DVE queue.
                    pjq = ps.tile([64, 128], f32, tag="pj", name=f"pjq{tt}")
                    for c in range(NC_):
                        nc.tensor.matmul(pjq[:], w_sb[:, 192 * c:192 * c + 64],
                                         xsrc(tt, c),
                                         start=(c == 0), stop=(c == NC_ - 1))
                    nc.scalar.activation(qT[:, 128 * tt:128 * (tt + 1)],
                                         pjq[:],
                                         mybir.ActivationFunctionType.Copy)
                    return
                wid = 192 if is_key else 64
                pj = ps.tile([128, wid], f32, tag="pj", name=f"pj{tt}")
                for c in range(NC_):
                    nc.tensor.matmul(pj[:], xsrc(tt, c),
                                     w_sb[:, 192 * c:192 * c + wid],
                                     start=(c == 0), stop=(c == NC_ - 1))
                qk = qkp.tile([128, 128 if is_key else 64], bf16, tag="qk",
                              name=f"qk{tt}")
                if is_key:
                    j = 2 * (tt // 4) + (tt & 1)
                    nc.vector.tensor_copy(qk[:], pj[:, 0:128])
                    veng = nc.scalar if tt < 20 else nc.vector
                    if tt < 20:
                        nc.scalar.activation(v_sb[:, 65 * j:65 * j + 64],
                                             pj[:, 128:192],
                                             mybir.ActivationFunctionType.Copy)
                    else:
                        nc.vector.tensor_copy(v_sb[:, 65 * j:65 * j + 64],
                                              pj[:, 128:192])
                else:
                    nc.vector.tensor_copy(qk[:], pj[:])
                tp = ps.tile([64, 256], bf16, tag="pj", name=f"tp{tt}")
                if proj_prev[0] is not None:
                    proj_tp(proj_prev[0])
                proj_prev[0] = (tt, qk, tp)

            def proj_flush():
                if proj_prev[0] is not None:
                    proj_tp(proj_prev[0])
                    proj_prev[0] = None

            # ---- attention: one continuous pair stream across supertiles ----
            # Global software pipeline: the U matmuls of a pair are deferred
            # two pairs (possibly crossing into the next supertile) so the
            # PE never sits on the ACT exp critical path, and the ACT stream
            # has no supertile-boundary bubble.
            u_state = {}   # st -> (u4 tile, n_emitted)
            pendings = []  # (st, jp, p2)

            def emit_u_d(st, jp, p2, d):
                if st not in u_state:
                    u_state[st] = [ps.tile([128, 260], f32, tag="u",
                                           name=f"u{st}", bufs=1), 0]
                ent = u_state[st]
                u4 = ent[0]
                j = 2 * jp + d
                for g in range(4):
                    # start=True zeroes the WHOLE PSUM bank: set it only
                    # on the chronologically first matmul into u4.
                    nc.tensor.matmul(
                        u4[:, 65 * g:65 * (g + 1)],
                        p2[:, 512 * d + 128 * g:512 * d + 128 * (g + 1)],
                        v_sb[:, 65 * j:65 * (j + 1)],
                        start=(ent[1] == 0),
                        stop=(ent[1] == 8 * (st + 1) - 1),
                        skip_group_check=True)
                    ent[1] += 1

            def ship(st):
                ent = u_state[st]
                if ent[1] == 8 * (st + 1):  # supertile complete -> ship it
                    uo_t = uop.tile([128, 260], f32, tag="uo", name=f"uo{st}")
                    if st <= 3:
                        nc.scalar.activation(uo_t[:], ent[0][:],
                                             mybir.ActivationFunctionType.Copy)
                    else:
                        nc.vector.tensor_copy(uo_t[:], ent[0][:])
                    eng = nc.sync if st == NST - 1 else nc.gpsimd
                    eng.dma_start(out[:, 260 * st:260 * (st + 1)], uo_t[:])
                    del u_state[st]

            def emit_u(st, jp, p2):
                emit_u_d(st, jp, p2, 0)
                emit_u_d(st, jp, p2, 1)
                ship(st)

            def emit_pair(st, jp):
                qsl = slice(STQ * st, STQ * (st + 1))
                s2 = ps.tile([128, 1024], f32, tag="s", name=f"s{st}_{jp}")
                p2 = pp.tile([128, 1024], bf16, tag="p", name=f"p{st}_{jp}")
                for d in range(2):
                    j = 2 * jp + d
                    nc.tensor.matmul(s2[:, 512 * d:512 * (d + 1)],
                                     kT[:, 128 * j:128 * (j + 1)],
                                     qT[:, qsl], start=True, stop=True)
                nc.scalar.activation(p2[:], s2[:],
                                     mybir.ActivationFunctionType.Exp,
                                     scale=0.125)
                if jp == st:  # diagonal pair -> causal mask
                    nc.vector.tensor_mul(p2[:], p2[:], msk_sb)
                pendings.append((st, jp, p2))
                if len(pendings) > 3:
                    emit_u(*pendings.pop(0))

            def filler_q(qtr, half):
                return deque(
                    (lambda t: (lambda: proj_unit(t)))(8 * qtr + 4 * half + i)
                    for i in range(4))

            # Fillers (projection units of supertile st+1) are emitted inside
            # supertile st's pair loop, starting at the pair index where their
            # DMA chunk has arrived (the stream is DMA-paced early on).
            for tl in range(4):
                proj_unit(tl)
            schedule = [filler_q(0, 1), filler_q(1, 0), filler_q(1, 1),
                        filler_q(2, 0), filler_q(2, 1), filler_q(3, 0),
                        filler_q(3, 1), deque()]
            for st in range(8):
                fillers = schedule[st]
                proj_flush()  # this supertile's Q^T/K^T must be complete
                for pi, jp in enumerate([st] + list(range(st))):  # diag first
                    emit_pair(st, jp)
                    if fillers:
                        fillers.popleft()()
                while fillers:
                    fillers.popleft()()
            for pd in pendings:
                emit_u(*pd)
            pendings.clear()

    nc.compile()
    return nc


def _get_nc():
    if "nc" not in _CACHE:
        _CACHE["nc"] = _build()
    return _CACHE["nc"]


def kernel(x, Wq, Wk, Wv, _trace=False):
    x = np.asarray(x)
    nc = _get_nc()

    # Token permutation per half: the program treats 256-token blocks with
    # block index ≡ 0 (mod 2) as key blocks.  For h=1 cores we swap each
    # even/odd 256-block pair so THEIR key blocks land on even positions.
    tok = np.arange(T)
    blk = tok // 256
    perm1 = np.where(blk % 2 == 0, tok + 256, tok - 256)  # swap block pairs

    xT = np.ascontiguousarray(x.transpose(0, 2, 1)).astype(BF)   # [B, C, T]
    xT1 = np.ascontiguousarray(xT[:, :, perm1])

    w_all = np.concatenate([Wq, Wk, Wv], axis=1).astype(np.float32)  # [C, 192]
    w_packed = np.ascontiguousarray(
        w_all.reshape(NC_, 128, 192).transpose(1, 0, 2).reshape(128, NC_ * 192)
    ).astype(BF)
    idn = np.eye(128, dtype=BF)

    # Masks for the diagonal pair (local key tiles (2st, 2st+1)); the causal
    # test runs on GLOBAL token ids, mapped through the per-half permutation.
    i = np.arange(STQ)[None, :]     # query offset within supertile (permuted)
    k = np.arange(128)[:, None]     # key offset within tile (permuted)
    masks = {}
    for h in range(2):
        if h == 0:
            gq = i
            gk = k
        else:
            gq = np.where(i < 256, i + 256, i - 256)
            gk = k + 256
        m = np.concatenate([(gk <= gq), (gk + 128 <= gq)], axis=0)  # [256,512]
        masks[h] = np.concatenate([m[0:128], m[128:256]],
                                  axis=1).astype(BF)  # [128, 1024]

    def pack_tiles(xTb):
        # [C, T] -> [128, tt*1024 + c*128 + t]
        return np.ascontiguousarray(
            xTb.reshape(NC_, 128, NTT, 128).transpose(1, 2, 0, 3)
            .reshape(128, NTT * 1024))

    in_maps = []
    for c in range(8):
        b, h = c % 4, c // 4
        xTb = xT[b] if h == 0 else xT1[b]
        in_maps.append({
            "xq": pack_tiles(xTb),
            "w": w_packed,
            "aux2": np.concatenate([masks[h], idn], axis=1),
        })

    res = bass_utils.run_bass_kernel_spmd(nc, in_maps, core_ids=list(range(8)),
                                          trace=_trace)
    _CACHE["last_results"] = res

    # Decode: U[c] [128, 8*260] -> [q_perm, 65]; un-permute h=1 tokens.
    O = np.empty((B, T, D), dtype=np.float32)
    for b in range(B):
        Uh = []
        for h in range(2):
            U = res.results[b + 4 * h]["out"]            # [128, 2080]
            U = U.reshape(128, NST, 4, 65).transpose(1, 2, 0, 3)
            U = U.reshape(T, 65)                          # permuted q order
            Uh.append(U[perm1] if h == 1 else U)          # global q order
        Ut = Uh[0] + Uh[1]
        O[b] = Ut[:, 0:64] / Ut[:, 64:65]
    return O


# revision 5
# speedup vs baseline: 1.0626x; 1.0308x over previous
"""Causal single-head attention (B=4, T=4096, C=1024, D=64) on 8 NeuronCores.

Sharding: core c = (batch b = c % 4, half h = c // 4).
Each core handles ALL queries of its batch against its half of the key
blocks (256-token blocks with block index ≡ h mod 2).  Pure SPMD: the
program is identical on every core; cores differ only in input data
(x[b]^T, block-pair-permuted for h=1, and the causal mask).  Each core
emits unnormalized U[q, 0:64] = sum_k exp(s) v and U[q, 64] = sum_k
exp(s); the host adds the two halves per batch and normalizes.

On-chip dataflow (bf16 compute, f32 PSUM accumulation):
  per 128-token tile tt:  PJ [128t, 192] = sum_c x_tile[c,tt]^T @ w[c]
    (fused Q|K|V projection, x tile stationary; non-key tiles: Q only)
  Q^T, K^T via PE transposes (deferred one unit for pipelining);
  V kept in [token, d] layout + ones column (softmax denominator)
  per query supertile st (512 q), local key tile pairs jp (diag first):
    S^T [128k, 2x512q] = K^T[j] @ Q^T[st]     (contraction over D=64)
    P^T = exp(S^T / 8)                        (one ACT instr per pair)
    diagonal pair: P^T *= mask                (DVE)
    U [128q, 65] += P^T-chunk^T @ [V_j | 1]   (P chunk stationary,
        deferred TWO pairs so the PE stays off the exp critical path)
  Projection units of the NEXT supertile interleave with attention
  pairs as fillers.  U -> SBUF -> DMA out per supertile (SWDGE/Pool
  path, keeping HWDGE free for input streaming).
"""
import sys
import numpy as np
import ml_dtypes
from collections import deque

if "/opt/trn_rl_repo" not in sys.path:
    sys.path.insert(0, "/opt/trn_rl_repo")

import concourse.bacc as bacc
import concourse.mybir as mybir
from concourse import tile
from concourse import bass_utils

bf16 = mybir.dt.bfloat16
f32 = mybir.dt.float32
BF = ml_dtypes.bfloat16

B, T, C, D = 4, 4096, 1024, 64
NC_ = C // 128      # 8 c-tiles
NTT = T // 128      # 32 token tiles
NST = 8             # query supertiles (512 q each)
STQ = 512

_CACHE = {}


def _build():
    nc = bacc.Bacc(None, target_bir_lowering=False, debug=False, num_devices=8)

    # xq tile-packed: xq[:, 1024*tt + 128*c : +128] = x^T[c-tile, token tile tt]
    xq = nc.dram_tensor("xq", [128, NTT * 1024], bf16, kind="ExternalInput")
    w = nc.dram_tensor("w", [128, NC_ * 192], bf16, kind="ExternalInput")
    # aux2 = diag mask [0:1024] | identity [1024:1152]
    aux2 = nc.dram_tensor("aux2", [128, 1152], bf16, kind="ExternalInput")
    out = nc.dram_tensor("out", [128, NST * 260], f32, kind="ExternalOutput")

    with tile.TileContext(nc) as tc:
        with tc.tile_pool(name="sb", bufs=1) as sb, \
             tc.tile_pool(name="qk", bufs=3) as qkp, \
             tc.tile_pool(name="pp", bufs=4) as pp, \
             tc.tile_pool(name="uo", bufs=2) as uop, \
             tc.tile_pool(name="ps", bufs=2, space="PSUM") as ps:

            # ---- resident tiles ----
            xq_sb = sb.tile([128, NTT * 1024], bf16, tag="xq")
            w_sb = sb.tile([128, NC_ * 192], bf16, tag="w")
            aux_sb = sb.tile([128, 1152], bf16, tag="aux")
            msk_sb = aux_sb[:, 0:1024]
            idn_sb = aux_sb[:, 1024:1152]
            qT = sb.tile([64, T], bf16, tag="qT")       # Q^T strip
            kT = sb.tile([64, T // 2], bf16, tag="kT")  # K^T local tiles
            v_sb = sb.tile([128, 16 * 65], bf16, tag="v")  # [V_j | 1] tiles

            # DMA order = consumption order: w, then one DMA per token tile
            # (each delivers ALL c-tiles of that tile, so projection unit tt
            # unblocks as soon as ITS chunk lands).
            nc.sync.dma_start(w_sb[:], w[:])
            for tt in range(NTT):
                nc.sync.dma_start(xq_sb[:, 1024 * tt:1024 * (tt + 1)],
                                  xq[:, 1024 * tt:1024 * (tt + 1)])
                if tt == 2:  # identity gates the first transposes (~chunk3)
                    nc.sync.dma_start(aux_sb[:], aux2[:])
            # ones columns of the V tiles (denominator trick)
            for j in range(16):
                nc.gpsimd.memset(v_sb[:, 65 * j + 64:65 * j + 65], 1.0)

            # PE p-state warmup: the PE ramps to full clock only after ~3us
            # of continuous busy; run junk matmuls on a zeroed tile right at
            # t=0 so the ramp completes before the first real projection.
            warm = sb.tile([128, 640], bf16, tag="warm")
            nc.vector.memset(warm[:], 0.0)
            # trigger the ACT Exp table load (1.28us) at t=0, not at the
            # first real exp
            nc.scalar.activation(warm[:, 0:1], warm[:, 0:1],
                                 mybir.ActivationFunctionType.Exp, scale=1.0)
            for i in range(5):
                wps = ps.tile([128, 512], f32, tag="s", name=f"warm{i}")
                nc.tensor.matmul(wps[:], warm[:, 0:128], warm[:, 128:640],
                                 start=True, stop=True)

            # ---- projection units, software-pipelined ----
            # mm-phase: fused projection matmuls into PSUM + DVE copies out.
            # tp-phase (transposes + Q^T/K^T copies) is deferred one unit so
            # the PE never waits on the just-issued DVE copy.
            proj_prev = [None]

            def xsrc(tt, c):
                return xq_sb[:, 1024 * tt + 128 * c:1024 * tt + 128 * (c + 1)]

            def proj_tp(state):
                tt, qk, tp = state
                is_key = ((tt >> 1) & 1) == 0
                if is_key:
                    j = 2 * (tt // 4) + (tt & 1)
                    nc.tensor.transpose(tp[:, 0:128], qk[:, 0:64], idn_sb)
                    nc.tensor.transpose(tp[:, 128:256], qk[:, 64:128], idn_sb)
                    nc.vector.tensor_copy(qT[:, 128 * tt:128 * (tt + 1)],
                                          tp[:, 0:128])
                    nc.vector.tensor_copy(kT[:, 128 * j:128 * (j + 1)],
                                          tp[:, 128:256])
                else:
                    nc.tensor.transpose(tp[:, 0:128], qk[:], idn_sb)
                    nc.vector.tensor_copy(qT[:, 128 * tt:128 * (tt + 1)],
                                          tp[:, 0:128])

            def proj_unit(tt):
                """Key-ness fixed to (tt//2)%2==0; h=1 cores get block-pair-
                permuted inputs so their key blocks land on even blocks."""
                is_key = ((tt >> 1) & 1) == 0
                if not is_key and tt % 4 == 3 and tt <= 15:
                    # The tile that gates attention(st): compute Q^T directly
                    # (w stationary), skipping the qk-copy + transpose hops,
                    # with the qT copy on ACT to dodge the DVE queue.
                    pjq = ps.tile([64, 128], f32, tag="pj", name=f"pjq{tt}")
                    for c in range(NC_):
                        nc.tensor.matmul(pjq[:], w_sb[:, 192 * c:192 * c + 64],
                                         xsrc(tt, c),
                                         start=(c == 0), stop=(c == NC_ - 1))
                    nc.scalar.activation(qT[:, 128 * tt:128 * (tt + 1)],
                                         pjq[:],
                                         mybir.ActivationFunctionType.Copy)
                    return
                wid = 192 if is_key else 64
                pj = ps.tile([128, wid], f32, tag="pj", name=f"pj{tt}")
                for c in range(NC_):
                    nc.tensor.matmul(pj[:], xsrc(tt, c),
                                     w_sb[:, 192 * c:192 * c + wid],
                                     start=(c == 0), stop=(c == NC_ - 1))
                qk = qkp.tile([128, 128 if is_key else 64], bf16, tag="qk",
                              name=f"qk{tt}")
                if is_key:
                    j = 2 * (tt // 4) + (tt & 1)
                    nc.vector.tensor_copy(qk[:], pj[:, 0:128])
                    nc.vector.tensor_copy(v_sb[:, 65 * j:65 * j + 64],
                                          pj[:, 128:192])
                else:
                    nc.vector.tensor_copy(qk[:], pj[:])
                tp = ps.tile([64, 256], bf16, tag="pj", name=f"tp{tt}")
                if proj_prev[0] is not None:
                    proj_tp(proj_prev[0])
                proj_prev[0] = (tt, qk, tp)

            def proj_flush():
                if proj_prev[0] is not None:
                    proj_tp(proj_prev[0])
                    proj_prev[0] = None

            # ---- attention: one continuous pair stream across supertiles ----
            # Global software pipeline: the U matmuls of a pair are deferred
            # two pairs (possibly crossing into the next supertile) so the
            # PE never sits on the ACT exp critical path, and the ACT stream
            # has no supertile-boundary bubble.
            u_state = {}   # st -> (u4 tile, n_emitted)
            pendings = []  # (st, jp, p2)

            def emit_u_d(st, jp, p2, d):
                if st not in u_state:
                    u_state[st] = [ps.tile([128, 260], f32, tag="u",
                                           name=f"u{st}", bufs=1), 0]
                ent = u_state[st]
                u4 = ent[0]
                j = 2 * jp + d
                for g in range(4):
                    # start=True zeroes the WHOLE PSUM bank: set it only
                    # on the chronologically first matmul into u4.
                    nc.tensor.matmul(
                        u4[:, 65 * g:65 * (g + 1)],
                        p2[:, 512 * d + 128 * g:512 * d + 128 * (g + 1)],
                        v_sb[:, 65 * j:65 * (j + 1)],
                        start=(ent[1] == 0),
                        stop=(ent[1] == 8 * (st + 1) - 1),
                        skip_group_check=True)
                    ent[1] += 1

            def ship(st):
                ent = u_state[st]
                if ent[1] == 8 * (st + 1):  # supertile complete -> ship it
                    uo_t = uop.tile([128, 260], f32, tag="uo", name=f"uo{st}")
                    if st <= 2:
                        nc.scalar.activation(uo_t[:], ent[0][:],
                                             mybir.ActivationFunctionType.Copy)
                    else:
                        nc.vector.tensor_copy(uo_t[:], ent[0][:])
                    eng = nc.sync if st == NST - 1 else nc.gpsimd
                    eng.dma_start(out[:, 260 * st:260 * (st + 1)], uo_t[:])
                    del u_state[st]

            def emit_u(st, jp, p2):
                if jp == st:  # diagonal pair -> causal mask, deferred here so
                    # the DVE is free for critical copies at the boundary
                    nc.vector.tensor_mul(p2[:], p2[:], msk_sb)
                emit_u_d(st, jp, p2, 0)
                emit_u_d(st, jp, p2, 1)
                ship(st)

            def emit_pair(st, jp):
                qsl = slice(STQ * st, STQ * (st + 1))
                s2 = ps.tile([128, 1024], f32, tag="s", name=f"s{st}_{jp}")
                p2 = pp.tile([128, 1024], bf16, tag="p", name=f"p{st}_{jp}")
                for d in range(2):
                    j = 2 * jp + d
                    nc.tensor.matmul(s2[:, 512 * d:512 * (d + 1)],
                                     kT[:, 128 * j:128 * (j + 1)],
                                     qT[:, qsl], start=True, stop=True)
                nc.scalar.activation(p2[:], s2[:],
                                     mybir.ActivationFunctionType.Exp,
                                     scale=0.125)
                pendings.append((st, jp, p2))
                if len(pendings) > 3:
                    emit_u(*pendings.pop(0))

            def filler_q(qtr, half):
                return deque(
                    (lambda t: (lambda: proj_unit(t)))(8 * qtr + 4 * half + i)
                    for i in range(4))

            # Fillers (projection units of supertile st+1) are emitted inside
            # supertile st's pair loop, starting at the pair index where their
            # DMA chunk has arrived (the stream is DMA-paced early on).
            for tl in range(4):
                proj_unit(tl)
            schedule = [filler_q(0, 1), filler_q(1, 0), filler_q(1, 1),
                        filler_q(2, 0), filler_q(2, 1), filler_q(3, 0),
                        filler_q(3, 1), deque()]
            for st in range(8):
                fillers = schedule[st]
                proj_flush()  # this supertile's Q^T/K^T must be complete
                for pi, jp in enumerate([st] + list(range(st))):  # diag first
                    emit_pair(st, jp)
                    if fillers:
                        fillers.popleft()()
                while fillers:
                    fillers.popleft()()
            for pd in pendings:
                emit_u(*pd)
            pendings.clear()

    nc.compile()
    return nc


def _get_nc():
    if "nc" not in _CACHE:
        _CACHE["nc"] = _build()
    return _CACHE["nc"]


def kernel(x, Wq, Wk, Wv, _trace=False):
    x = np.asarray(x)
    nc = _get_nc()

    # Token permutation per half: the program treats 256-token blocks with
    # block index ≡ 0 (mod 2) as key blocks.  For h=1 cores we swap each
    # even/odd 256-block pair so THEIR key blocks land on even positions.
    tok = np.arange(T)
    blk = tok // 256
    perm1 = np.where(blk % 2 == 0, tok + 256, tok - 256)  # swap block pairs

    xT = np.ascontiguousarray(x.transpose(0, 2, 1)).astype(BF)   # [B, C, T]
    xT1 = np.ascontiguousarray(xT[:, :, perm1])

    w_all = np.concatenate([Wq, Wk, Wv], axis=1).astype(np.float32)  # [C, 192]
    w_packed = np.ascontiguousarray(
        w_all.reshape(NC_, 128, 192).transpose(1, 0, 2).reshape(128, NC_ * 192)
    ).astype(BF)
    idn = np.eye(128, dtype=BF)

    # Masks for the diagonal pair (local key tiles (2st, 2st+1)); the causal
    # test runs on GLOBAL token ids, mapped through the per-half permutation.
    i = np.arange(STQ)[None, :]     # query offset within supertile (permuted)
    k = np.arange(128)[:, None]     # key offset within tile (permuted)
    masks = {}
    for h in range(2):
        if h == 0:
            gq = i
            gk = k
        else:
            gq = np.where(i < 256, i + 256, i - 256)
            gk = k + 256
        m = np.concatenate([(gk <= gq), (gk + 128 <= gq)], axis=0)  # [256,512]
        masks[h] = np.concatenate([m[0:128], m[128:256]],
                                  axis=1).astype(BF)  # [128, 1024]

    def pack_tiles(xTb):
        # [C, T] -> [128, tt*1024 + c*128 + t]
        return np.ascontiguousarray(
            xTb.reshape(NC_, 128, NTT, 128).transpose(1, 2, 0, 3)
            .reshape(128, NTT * 1024))

    in_maps = []
    for c in range(8):
        b, h = c % 4, c // 4
        xTb = xT[b] if h == 0 else xT1[b]
        in_maps.append({
            "xq": pack_tiles(xTb),
            "w": w_packed,
            "aux2": np.concatenate([masks[h], idn], axis=1),
        })

    res = bass_utils.run_bass_kernel_spmd(nc, in_maps, core_ids=list(range(8)),
                                          trace=_trace)
    _CACHE["last_results"] = res

    # Decode: U[c] [128, 8*260] -> [q_perm, 65]; un-permute h=1 tokens.
    O = np.empty((B, T, D), dtype=np.float32)
    for b in range(B):
        Uh = []
        for h in range(2):
            U = res.results[b + 4 * h]["out"]            # [128, 2080]
            U = U.reshape(128, NST, 4, 65).transpose(1, 2, 0, 3)
            U = U.reshape(T, 65)                          # permuted q order
            Uh.append(U[perm1] if h == 1 else U)          # global q order
        Ut = Uh[0] + Uh[1]
        O[b] = Ut[:, 0:64] / Ut[:, 64:65]
    return O


# revision 6
# speedup vs baseline: 1.0810x; 1.0173x over previous
"""Causal single-head attention (B=4, T=4096, C=1024, D=64) on 8 NeuronCores.

Sharding: core c = (batch b = c % 4, half h = c // 4).
Each core handles ALL queries of its batch against its half of the key
blocks (256-token blocks with block index ≡ h mod 2).  Pure SPMD: the
program is identical on every core; cores differ only in input data
(x[b]^T, block-pair-permuted for h=1, and the causal mask).  Each core
emits unnormalized U[q, 0:64] = sum_k exp(s) v and U[q, 64] = sum_k
exp(s); the host adds the two halves per batch and normalizes.

On-chip dataflow (bf16 compute, f32 PSUM accumulation):
  per 128-token tile tt:  PJ [128t, 192] = sum_c x_tile[c,tt]^T @ w[c]
    (fused Q|K|V projection, x tile stationary; non-key tiles: Q only)
  Q^T, K^T via PE transposes (deferred one unit for pipelining);
  V kept in [token, d] layout + ones column (softmax denominator)
  per query supertile st (512 q), local key tile pairs jp (diag first):
    S^T [128k, 2x512q] = K^T[j] @ Q^T[st]     (contraction over D=64)
    P^T = exp(S^T / 8)                        (one ACT instr per pair)
    diagonal pair: P^T *= mask                (DVE)
    U [128q, 65] += P^T-chunk^T @ [V_j | 1]   (P chunk stationary,
        deferred TWO pairs so the PE stays off the exp critical path)
  Projection units of the NEXT supertile interleave with attention
  pairs as fillers.  U -> SBUF -> DMA out per supertile (SWDGE/Pool
  path, keeping HWDGE free for input streaming).
"""
import sys
import numpy as np
import ml_dtypes
from collections import deque

if "/opt/trn_rl_repo" not in sys.path:
    sys.path.insert(0, "/opt/trn_rl_repo")

import concourse.bacc as bacc
import concourse.mybir as mybir
from concourse import tile
from concourse import bass_utils

bf16 = mybir.dt.bfloat16
f32 = mybir.dt.float32
BF = ml_dtypes.bfloat16

B, T, C, D = 4, 4096, 1024, 64
NC_ = C // 128      # 8 c-tiles
NTT = T // 128      # 32 token tiles
NST = 8             # query supertiles (512 q each)
STQ = 512

_CACHE = {}


def _build():
    nc = bacc.Bacc(None, target_bir_lowering=False, debug=False, num_devices=8)

    # xq tile-packed: xq[:, 1024*tt + 128*c : +128] = x^T[c-tile, token tile tt]
    xq = nc.dram_tensor("xq", [128, NTT * 1024], bf16, kind="ExternalInput")
    w = nc.dram_tensor("w", [128, NC_ * 192], bf16, kind="ExternalInput")
    # aux2 = diag mask [0:768] | identity [768:896]
    aux2 = nc.dram_tensor("aux2", [128, 896], bf16, kind="ExternalInput")
    out = nc.dram_tensor("out", [128, NST * 260], f32, kind="ExternalOutput")

    with tile.TileContext(nc) as tc:
        with tc.tile_pool(name="sb", bufs=1) as sb, \
             tc.tile_pool(name="qk", bufs=3) as qkp, \
             tc.tile_pool(name="pp", bufs=4) as pp, \
             tc.tile_pool(name="uo", bufs=2) as uop, \
             tc.tile_pool(name="ps", bufs=2, space="PSUM") as ps:

            # ---- resident tiles ----
            xq_sb = sb.tile([128, NTT * 1024], bf16, tag="xq")
            w_sb = sb.tile([128, NC_ * 192], bf16, tag="w")
            aux_sb = sb.tile([128, 896], bf16, tag="aux")
            msk_sb = aux_sb[:, 0:768]
            idn_sb = aux_sb[:, 768:896]
            qT = sb.tile([64, T], bf16, tag="qT")       # Q^T strip
            kT = sb.tile([64, T // 2], bf16, tag="kT")  # K^T local tiles
            v_sb = sb.tile([128, 16 * 65], bf16, tag="v")  # [V_j | 1] tiles

            # DMA order = consumption order: w, then one DMA per token tile
            # (each delivers ALL c-tiles of that tile, so projection unit tt
            # unblocks as soon as ITS chunk lands).
            nc.sync.dma_start(w_sb[:], w[:])
            for tt in range(NTT):
                nc.sync.dma_start(xq_sb[:, 1024 * tt:1024 * (tt + 1)],
                                  xq[:, 1024 * tt:1024 * (tt + 1)])
                if tt == 2:  # identity gates the first transposes (~chunk3)
                    nc.sync.dma_start(aux_sb[:], aux2[:])
            # ones columns of the V tiles (denominator trick)
            for j in range(16):
                nc.gpsimd.memset(v_sb[:, 65 * j + 64:65 * j + 65], 1.0)

            # PE p-state warmup: the PE ramps to full clock only after ~3us
            # of continuous busy; run junk matmuls on a zeroed tile right at
            # t=0 so the ramp completes before the first real projection.
            warm = sb.tile([128, 640], bf16, tag="warm")
            nc.vector.memset(warm[:], 0.0)
            # trigger the ACT Exp table load (1.28us) at t=0, not at the
            # first real exp
            nc.scalar.activation(warm[:, 0:1], warm[:, 0:1],
                                 mybir.ActivationFunctionType.Exp, scale=1.0)
            for i in range(5):
                wps = ps.tile([128, 512], f32, tag="s", name=f"warm{i}")
                nc.tensor.matmul(wps[:], warm[:, 0:128], warm[:, 128:640],
                                 start=True, stop=True)

            # ---- projection units, software-pipelined ----
            # mm-phase: fused projection matmuls into PSUM + DVE copies out.
            # tp-phase (transposes + Q^T/K^T copies) is deferred one unit so
            # the PE never waits on the just-issued DVE copy.
            proj_prev = [None]

            def xsrc(tt, c):
                return xq_sb[:, 1024 * tt + 128 * c:1024 * tt + 128 * (c + 1)]

            def proj_tp(state):
                tt, qk, tp = state
                is_key = (tt & 1) == 0
                if is_key:
                    j = tt // 2
                    nc.tensor.transpose(tp[:, 0:128], qk[:, 0:64], idn_sb)
                    nc.tensor.transpose(tp[:, 128:256], qk[:, 64:128], idn_sb)
                    nc.vector.tensor_copy(qT[:, 128 * tt:128 * (tt + 1)],
                                          tp[:, 0:128])
                    nc.vector.tensor_copy(kT[:, 128 * j:128 * (j + 1)],
                                          tp[:, 128:256])
                else:
                    nc.tensor.transpose(tp[:, 0:128], qk[:], idn_sb)
                    nc.vector.tensor_copy(qT[:, 128 * tt:128 * (tt + 1)],
                                          tp[:, 0:128])

            def proj_unit(tt):
                """Key-ness fixed to tt%2==0 (128-interleaved); h=1 cores get
                adjacent-tile-permuted inputs so their key tiles land on
                even positions."""
                is_key = (tt & 1) == 0
                if not is_key and tt % 4 == 3 and tt <= 19:
                    # The tile that gates attention(st): compute Q^T directly
                    # (w stationary), skipping the qk-copy + transpose hops,
                    # with the qT copy on ACT to dodge the DVE queue.
                    pjq = ps.tile([64, 128], f32, tag="pj", name=f"pjq{tt}")
                    for c in range(NC_):
                        nc.tensor.matmul(pjq[:], w_sb[:, 192 * c:192 * c + 64],
                                         xsrc(tt, c),
                                         start=(c == 0), stop=(c == NC_ - 1))
                    nc.scalar.activation(qT[:, 128 * tt:128 * (tt + 1)],
                                         pjq[:],
                                         mybir.ActivationFunctionType.Copy)
                    return
                wid = 192 if is_key else 64
                pj = ps.tile([128, wid], f32, tag="pj", name=f"pj{tt}")
                for c in range(NC_):
                    nc.tensor.matmul(pj[:], xsrc(tt, c),
                                     w_sb[:, 192 * c:192 * c + wid],
                                     start=(c == 0), stop=(c == NC_ - 1))
                qk = qkp.tile([128, 128 if is_key else 64], bf16, tag="qk",
                              name=f"qk{tt}")
                if is_key:
                    j = tt // 2
                    nc.vector.tensor_copy(qk[:], pj[:, 0:128])
                    nc.vector.tensor_copy(v_sb[:, 65 * j:65 * j + 64],
                                          pj[:, 128:192])
                else:
                    nc.vector.tensor_copy(qk[:], pj[:])
                tp = ps.tile([64, 256], bf16, tag="pj", name=f"tp{tt}")
                if proj_prev[0] is not None:
                    proj_tp(proj_prev[0])
                proj_prev[0] = (tt, qk, tp)

            def proj_flush():
                if proj_prev[0] is not None:
                    proj_tp(proj_prev[0])
                    proj_prev[0] = None

            # ---- attention: one continuous pair stream across supertiles ----
            # Global software pipeline: the U matmuls of a pair are deferred
            # two pairs (possibly crossing into the next supertile) so the
            # PE never sits on the ACT exp critical path, and the ACT stream
            # has no supertile-boundary bubble.
            u_state = {}   # st -> (u4 tile, n_emitted)
            pendings = []  # (st, jp, p2)

            def emit_u_d(st, jp, p2, d):
                if st not in u_state:
                    u_state[st] = [ps.tile([128, 260], f32, tag="u",
                                           name=f"u{st}", bufs=1), 0]
                ent = u_state[st]
                u4 = ent[0]
                j = 2 * jp + d
                total = 8 * (st + 1) - 2  # diag d1 contributes only g=2,3
                if jp == st and d == 1:
                    gs = [(2, 512), (3, 640)]
                else:
                    gs = [(g, 512 * d + 128 * g) for g in range(4)]
                for g, lo in gs:
                    # start=True zeroes the WHOLE PSUM bank: set it only
                    # on the chronologically first matmul into u4.
                    nc.tensor.matmul(
                        u4[:, 65 * g:65 * (g + 1)],
                        p2[:, lo:lo + 128],
                        v_sb[:, 65 * j:65 * (j + 1)],
                        start=(ent[1] == 0),
                        stop=(ent[1] == total - 1),
                        skip_group_check=True)
                    ent[1] += 1

            def ship(st):
                ent = u_state[st]
                if ent[1] == 8 * (st + 1) - 2:  # supertile complete -> ship
                    uo_t = uop.tile([128, 260], f32, tag="uo", name=f"uo{st}")
                    if st <= 2:
                        nc.scalar.activation(uo_t[:], ent[0][:],
                                             mybir.ActivationFunctionType.Copy)
                    else:
                        nc.vector.tensor_copy(uo_t[:], ent[0][:])
                    eng = nc.sync if st == NST - 1 else nc.gpsimd
                    eng.dma_start(out[:, 260 * st:260 * (st + 1)], uo_t[:])
                    del u_state[st]

            def emit_u(st, jp, p2):
                if jp == st:  # diagonal pair -> causal mask, deferred here so
                    # the DVE is free for critical copies at the boundary
                    nc.vector.tensor_mul(p2[:, 0:768], p2[:, 0:768], msk_sb)
                emit_u_d(st, jp, p2, 0)
                emit_u_d(st, jp, p2, 1)
                ship(st)

            def emit_pair(st, jp):
                qsl = slice(STQ * st, STQ * (st + 1))
                s2 = ps.tile([128, 1024], f32, tag="s", name=f"s{st}_{jp}")
                p2 = pp.tile([128, 1024], bf16, tag="p", name=f"p{st}_{jp}")
                if jp == st:
                    # diagonal pair: with 128-interleaved keys, the second
                    # tile is visible only to queries [256:512) for BOTH
                    # halves -> 768 live columns instead of 1024
                    nc.tensor.matmul(s2[:, 0:512],
                                     kT[:, 128 * 2 * jp:128 * (2 * jp + 1)],
                                     qT[:, qsl], start=True, stop=True)
                    nc.tensor.matmul(s2[:, 512:768],
                                     kT[:, 128 * (2 * jp + 1):128 * (2 * jp + 2)],
                                     qT[:, STQ * st + 256:STQ * (st + 1)],
                                     start=True, stop=True)
                    nc.scalar.activation(p2[:, 0:768], s2[:, 0:768],
                                         mybir.ActivationFunctionType.Exp,
                                         scale=0.125)
                else:
                    for d in range(2):
                        j = 2 * jp + d
                        nc.tensor.matmul(s2[:, 512 * d:512 * (d + 1)],
                                         kT[:, 128 * j:128 * (j + 1)],
                                         qT[:, qsl], start=True, stop=True)
                    nc.scalar.activation(p2[:], s2[:],
                                         mybir.ActivationFunctionType.Exp,
                                         scale=0.125)
                pendings.append((st, jp, p2))
                if len(pendings) > 3:
                    emit_u(*pendings.pop(0))

            def filler_q(qtr, half):
                return deque(
                    (lambda t: (lambda: proj_unit(t)))(8 * qtr + 4 * half + i)
                    for i in range(4))

            # Fillers (projection units of supertile st+1) are emitted inside
            # supertile st's pair loop, starting at the pair index where their
            # DMA chunk has arrived (the stream is DMA-paced early on).
            for tl in range(4):
                proj_unit(tl)
            schedule = [filler_q(0, 1), filler_q(1, 0), filler_q(1, 1),
                        filler_q(2, 0), filler_q(2, 1), filler_q(3, 0),
                        filler_q(3, 1), deque()]
            for st in range(8):
                fillers = schedule[st]
                proj_flush()  # this supertile's Q^T/K^T must be complete
                for pi, jp in enumerate([st] + list(range(st))):  # diag first
                    emit_pair(st, jp)
                    if fillers:
                        fillers.popleft()()
                while fillers:
                    fillers.popleft()()
            for pd in pendings:
                emit_u(*pd)
            pendings.clear()

    nc.compile()
    return nc


def _get_nc():
    if "nc" not in _CACHE:
        _CACHE["nc"] = _build()
    return _CACHE["nc"]


def kernel(x, Wq, Wk, Wv, _trace=False):
    x = np.asarray(x)
    nc = _get_nc()

    # Token permutation per half: the program treats EVEN 128-token tiles
    # as key tiles.  For h=1 cores we swap each adjacent tile pair so THEIR
    # key tiles land on even positions.
    tok = np.arange(T)
    perm1 = 128 * ((tok // 128) ^ 1) + tok % 128  # swap adjacent 128-tiles

    xT = np.ascontiguousarray(x.transpose(0, 2, 1)).astype(BF)   # [B, C, T]
    xT1 = np.ascontiguousarray(xT[:, :, perm1])

    w_all = np.concatenate([Wq, Wk, Wv], axis=1).astype(np.float32)  # [C, 192]
    w_packed = np.ascontiguousarray(
        w_all.reshape(NC_, 128, 192).transpose(1, 0, 2).reshape(128, NC_ * 192)
    ).astype(BF)
    idn = np.eye(128, dtype=BF)

    # Masks for the diagonal pair: program key tile A holds global tile
    # 4st+h, tile B holds 4st+2+h; program query quarter g holds global
    # tile 4st+(g^h).  Causal test on global ids:
    #   A: 128h + k <= 128(g^h) + i   (cols 0:512, all four quarters)
    #   B: 128(2+h) + k <= 128(g^h) + i   (cols 512:768, quarters g=2,3)
    i = np.arange(128)[None, :]
    k = np.arange(128)[:, None]
    masks = {}
    for h in range(2):
        colsA = [(128 * h + k <= 128 * (g ^ h) + i) for g in range(4)]
        colsB = [(128 * (2 + h) + k <= 128 * (g ^ h) + i) for g in (2, 3)]
        masks[h] = np.concatenate(colsA + colsB, axis=1).astype(BF)  # [128,768]

    def pack_tiles(xTb):
        # [C, T] -> [128, tt*1024 + c*128 + t]
        return np.ascontiguousarray(
            xTb.reshape(NC_, 128, NTT, 128).transpose(1, 2, 0, 3)
            .reshape(128, NTT * 1024))

    in_maps = []
    for c in range(8):
        b, h = c % 4, c // 4
        xTb = xT[b] if h == 0 else xT1[b]
        in_maps.append({
            "xq": pack_tiles(xTb),
            "w": w_packed,
            "aux2": np.concatenate([masks[h], idn], axis=1),
        })

    res = bass_utils.run_bass_kernel_spmd(nc, in_maps, core_ids=list(range(8)),
                                          trace=_trace)
    _CACHE["last_results"] = res

    # Decode: U[c] [128, 8*260] -> [q_perm, 65]; un-permute h=1 tokens.
    O = np.empty((B, T, D), dtype=np.float32)
    for b in range(B):
        Uh = []
        for h in range(2):
            U = res.results[b + 4 * h]["out"]            # [128, 2080]
            U = U.reshape(128, NST, 4, 65).transpose(1, 2, 0, 3)
            U = U.reshape(T, 65)                          # permuted q order
            Uh.append(U[perm1] if h == 1 else U)          # global q order
        Ut = Uh[0] + Uh[1]
        O[b] = Ut[:, 0:64] / Ut[:, 64:65]
    return O


# revision 7
# speedup vs baseline: 1.0959x; 1.0138x over previous
"""Causal single-head attention (B=4, T=4096, C=1024, D=64) on 8 NeuronCores.

Sharding: core c = (batch b = c % 4, half h = c // 4).
Each core handles ALL queries of its batch against its half of the key
blocks (256-token blocks with block index ≡ h mod 2).  Pure SPMD: the
program is identical on every core; cores differ only in input data
(x[b]^T, block-pair-permuted for h=1, and the causal mask).  Each core
emits unnormalized U[q, 0:64] = sum_k exp(s) v and U[q, 64] = sum_k
exp(s); the host adds the two halves per batch and normalizes.

On-chip dataflow (bf16 compute, f32 PSUM accumulation):
  per 128-token tile tt:  PJ [128t, 192] = sum_c x_tile[c,tt]^T @ w[c]
    (fused Q|K|V projection, x tile stationary; non-key tiles: Q only)
  Q^T, K^T via PE transposes (deferred one unit for pipelining);
  V kept in [token, d] layout + ones column (softmax denominator)
  per query supertile st (512 q), local key tile pairs jp (diag first):
    S^T [128k, 2x512q] = K^T[j] @ Q^T[st]     (contraction over D=64)
    P^T = exp(S^T / 8)                        (one ACT instr per pair)
    diagonal pair: P^T *= mask                (DVE)
    U [128q, 65] += P^T-chunk^T @ [V_j | 1]   (P chunk stationary,
        deferred TWO pairs so the PE stays off the exp critical path)
  Projection units of the NEXT supertile interleave with attention
  pairs as fillers.  U -> SBUF -> DMA out per supertile (SWDGE/Pool
  path, keeping HWDGE free for input streaming).
"""
import sys
import numpy as np
import ml_dtypes
from collections import deque

if "/opt/trn_rl_repo" not in sys.path:
    sys.path.insert(0, "/opt/trn_rl_repo")

import concourse.bacc as bacc
import concourse.mybir as mybir
from concourse import tile
from concourse import bass_utils

bf16 = mybir.dt.bfloat16
f32 = mybir.dt.float32
BF = ml_dtypes.bfloat16

B, T, C, D = 4, 4096, 1024, 64
NC_ = C // 128      # 8 c-tiles
NTT = T // 128      # 32 token tiles
NST = 8             # query supertiles (512 q each)
STQ = 512

_CACHE = {}


def _build():
    nc = bacc.Bacc(None, target_bir_lowering=False, debug=False, num_devices=8)

    # xq tile-packed: xq[:, 1024*tt + 128*c : +128] = x^T[c-tile, token tile tt]
    xq = nc.dram_tensor("xq", [128, NTT * 1024], bf16, kind="ExternalInput")
    w = nc.dram_tensor("w", [128, NC_ * 192], bf16, kind="ExternalInput")
    # aux2 = diag mask [0:768] | identity [768:896]
    aux2 = nc.dram_tensor("aux2", [128, 896], bf16, kind="ExternalInput")
    out = nc.dram_tensor("out", [128, NST * 260], f32, kind="ExternalOutput")

    with tile.TileContext(nc) as tc:
        with tc.tile_pool(name="sb", bufs=1) as sb, \
             tc.tile_pool(name="qk", bufs=3) as qkp, \
             tc.tile_pool(name="pp", bufs=4) as pp, \
             tc.tile_pool(name="uo", bufs=2) as uop, \
             tc.tile_pool(name="ps", bufs=2, space="PSUM") as ps:

            # ---- resident tiles ----
            xq_sb = sb.tile([128, NTT * 1024], bf16, tag="xq")
            w_sb = sb.tile([128, NC_ * 192], bf16, tag="w")
            aux_sb = sb.tile([128, 896], bf16, tag="aux")
            msk_sb = aux_sb[:, 0:768]
            idn_sb = aux_sb[:, 768:896]
            qT = sb.tile([64, T], bf16, tag="qT")       # Q^T strip
            kT = sb.tile([64, T // 2], bf16, tag="kT")  # K^T local tiles
            v_sb = sb.tile([128, 16 * 65], bf16, tag="v")  # [V_j | 1] tiles

            # DMA order = consumption order: w, then one DMA per token tile
            # (each delivers ALL c-tiles of that tile, so projection unit tt
            # unblocks as soon as ITS chunk lands).
            nc.sync.dma_start(w_sb[:], w[:])
            for tt in range(NTT):
                nc.sync.dma_start(xq_sb[:, 1024 * tt:1024 * (tt + 1)],
                                  xq[:, 1024 * tt:1024 * (tt + 1)])
                if tt == 2:  # identity gates the first transposes (~chunk3)
                    nc.sync.dma_start(aux_sb[:], aux2[:])
            # ones columns of the V tiles (denominator trick)
            for j in range(16):
                nc.gpsimd.memset(v_sb[:, 65 * j + 64:65 * j + 65], 1.0)

            # PE p-state warmup: the PE ramps to full clock only after ~3us
            # of continuous busy; run junk matmuls on a zeroed tile right at
            # t=0 so the ramp completes before the first real projection.
            warm = sb.tile([128, 640], bf16, tag="warm")
            nc.vector.memset(warm[:], 0.0)
            # trigger the ACT Exp table load (1.28us) at t=0, not at the
            # first real exp
            nc.scalar.activation(warm[:, 0:1], warm[:, 0:1],
                                 mybir.ActivationFunctionType.Exp, scale=1.0)
            for i in range(5):
                wps = ps.tile([128, 512], f32, tag="s", name=f"warm{i}")
                nc.tensor.matmul(wps[:], warm[:, 0:128], warm[:, 128:640],
                                 start=True, stop=True)

            # ---- projection units, software-pipelined ----
            # mm-phase: fused projection matmuls into PSUM + DVE copies out.
            # tp-phase (transposes + Q^T/K^T copies) is deferred one unit so
            # the PE never waits on the just-issued DVE copy.
            proj_prev = [None]

            def xsrc(tt, c):
                return xq_sb[:, 1024 * tt + 128 * c:1024 * tt + 128 * (c + 1)]

            def proj_tp(state):
                tt, qk, tp = state
                is_key = (tt & 1) == 0
                if is_key:
                    j = tt // 2
                    nc.tensor.transpose(tp[:, 0:128], qk[:, 0:64], idn_sb)
                    nc.tensor.transpose(tp[:, 128:256], qk[:, 64:128], idn_sb)
                    nc.vector.tensor_copy(qT[:, 128 * tt:128 * (tt + 1)],
                                          tp[:, 0:128])
                    nc.vector.tensor_copy(kT[:, 128 * j:128 * (j + 1)],
                                          tp[:, 128:256])
                else:
                    nc.tensor.transpose(tp[:, 0:128], qk[:], idn_sb)
                    nc.vector.tensor_copy(qT[:, 128 * tt:128 * (tt + 1)],
                                          tp[:, 0:128])

            def proj_unit(tt):
                """Key-ness fixed to tt%2==0 (128-interleaved); h=1 cores get
                adjacent-tile-permuted inputs so their key tiles land on
                even positions."""
                is_key = (tt & 1) == 0
                if not is_key and tt % 2 == 1 and tt <= 19:
                    # The tile that gates attention(st): compute Q^T directly
                    # (w stationary), skipping the qk-copy + transpose hops,
                    # with the qT copy on ACT to dodge the DVE queue.
                    pjq = ps.tile([64, 128], f32, tag="pj", name=f"pjq{tt}")
                    for c in range(NC_):
                        nc.tensor.matmul(pjq[:], w_sb[:, 192 * c:192 * c + 64],
                                         xsrc(tt, c),
                                         start=(c == 0), stop=(c == NC_ - 1))
                    nc.scalar.activation(qT[:, 128 * tt:128 * (tt + 1)],
                                         pjq[:],
                                         mybir.ActivationFunctionType.Copy)
                    return
                wid = 192 if is_key else 64
                pj = ps.tile([128, wid], f32, tag="pj", name=f"pj{tt}")
                for c in range(NC_):
                    nc.tensor.matmul(pj[:], xsrc(tt, c),
                                     w_sb[:, 192 * c:192 * c + wid],
                                     start=(c == 0), stop=(c == NC_ - 1))
                qk = qkp.tile([128, 128 if is_key else 64], bf16, tag="qk",
                              name=f"qk{tt}")
                if is_key:
                    j = tt // 2
                    nc.vector.tensor_copy(qk[:], pj[:, 0:128])
                    nc.vector.tensor_copy(v_sb[:, 65 * j:65 * j + 64],
                                          pj[:, 128:192])
                else:
                    nc.vector.tensor_copy(qk[:], pj[:])
                tp = ps.tile([64, 256], bf16, tag="pj", name=f"tp{tt}")
                if proj_prev[0] is not None:
                    proj_tp(proj_prev[0])
                proj_prev[0] = (tt, qk, tp)

            def proj_flush():
                if proj_prev[0] is not None:
                    proj_tp(proj_prev[0])
                    proj_prev[0] = None

            # ---- attention: one continuous pair stream across supertiles ----
            # Global software pipeline: the U matmuls of a pair are deferred
            # two pairs (possibly crossing into the next supertile) so the
            # PE never sits on the ACT exp critical path, and the ACT stream
            # has no supertile-boundary bubble.
            u_state = {}   # st -> (u4 tile, n_emitted)
            pendings = []  # (st, jp, p2)

            def emit_u_d(st, jp, p2, d):
                if st not in u_state:
                    u_state[st] = [ps.tile([128, 260], f32, tag="u",
                                           name=f"u{st}", bufs=1), 0]
                ent = u_state[st]
                u4 = ent[0]
                j = 2 * jp + d
                total = 8 * (st + 1) - 2  # diag d1 contributes only g=2,3
                if jp == st and d == 1:
                    gs = [(2, 512), (3, 640)]
                else:
                    gs = [(g, 512 * d + 128 * g) for g in range(4)]
                for g, lo in gs:
                    # start=True zeroes the WHOLE PSUM bank: set it only
                    # on the chronologically first matmul into u4.
                    nc.tensor.matmul(
                        u4[:, 65 * g:65 * (g + 1)],
                        p2[:, lo:lo + 128],
                        v_sb[:, 65 * j:65 * (j + 1)],
                        start=(ent[1] == 0),
                        stop=(ent[1] == total - 1),
                        skip_group_check=True)
                    ent[1] += 1

            def ship(st):
                ent = u_state[st]
                if ent[1] == 8 * (st + 1) - 2:  # supertile complete -> ship
                    uo_t = uop.tile([128, 260], f32, tag="uo", name=f"uo{st}")
                    if st <= 2:
                        nc.scalar.activation(uo_t[:], ent[0][:],
                                             mybir.ActivationFunctionType.Copy)
                    else:
                        nc.vector.tensor_copy(uo_t[:], ent[0][:])
                    eng = nc.sync if st == NST - 1 else nc.gpsimd
                    eng.dma_start(out[:, 260 * st:260 * (st + 1)], uo_t[:])
                    del u_state[st]

            def emit_u(st, jp, p2):
                if jp == st:  # diagonal pair -> causal mask, deferred here so
                    # the DVE is free for critical copies at the boundary
                    nc.vector.tensor_mul(p2[:, 0:768], p2[:, 0:768], msk_sb)
                emit_u_d(st, jp, p2, 0)
                emit_u_d(st, jp, p2, 1)
                ship(st)

            def emit_pair(st, jp):
                qsl = slice(STQ * st, STQ * (st + 1))
                s2 = ps.tile([128, 1024], f32, tag="s", name=f"s{st}_{jp}")
                p2 = pp.tile([128, 1024], bf16, tag="p", name=f"p{st}_{jp}")
                if jp == st:
                    # diagonal pair: with 128-interleaved keys, the second
                    # tile is visible only to queries [256:512) for BOTH
                    # halves -> 768 live columns instead of 1024
                    nc.tensor.matmul(s2[:, 0:512],
                                     kT[:, 128 * 2 * jp:128 * (2 * jp + 1)],
                                     qT[:, qsl], start=True, stop=True)
                    nc.tensor.matmul(s2[:, 512:768],
                                     kT[:, 128 * (2 * jp + 1):128 * (2 * jp + 2)],
                                     qT[:, STQ * st + 256:STQ * (st + 1)],
                                     start=True, stop=True)
                    nc.scalar.activation(p2[:, 0:768], s2[:, 0:768],
                                         mybir.ActivationFunctionType.Exp,
                                         scale=0.125)
                else:
                    for d in range(2):
                        j = 2 * jp + d
                        nc.tensor.matmul(s2[:, 512 * d:512 * (d + 1)],
                                         kT[:, 128 * j:128 * (j + 1)],
                                         qT[:, qsl], start=True, stop=True)
                    nc.scalar.activation(p2[:], s2[:],
                                         mybir.ActivationFunctionType.Exp,
                                         scale=0.125)
                pendings.append((st, jp, p2))
                if len(pendings) > 3:
                    emit_u(*pendings.pop(0))

            def filler_q(qtr, half):
                return deque(
                    (lambda t: (lambda: proj_unit(t)))(8 * qtr + 4 * half + i)
                    for i in range(4))

            # Fillers (projection units of supertile st+1) are emitted inside
            # supertile st's pair loop, starting at the pair index where their
            # DMA chunk has arrived (the stream is DMA-paced early on).
            for tl in range(4):
                proj_unit(tl)
            schedule = [filler_q(0, 1), filler_q(1, 0), filler_q(1, 1),
                        filler_q(2, 0), filler_q(2, 1), filler_q(3, 0),
                        filler_q(3, 1), deque()]
            for st in range(8):
                fillers = schedule[st]
                proj_flush()  # this supertile's Q^T/K^T must be complete
                for pi, jp in enumerate([st] + list(range(st))):  # diag first
                    emit_pair(st, jp)
                    if fillers:
                        fillers.popleft()()
                while fillers:
                    fillers.popleft()()
            for pd in pendings:
                emit_u(*pd)
            pendings.clear()

    nc.compile()
    return nc


def _get_nc():
    if "nc" not in _CACHE:
        _CACHE["nc"] = _build()
    return _CACHE["nc"]


def kernel(x, Wq, Wk, Wv, _trace=False):
    x = np.asarray(x)
    nc = _get_nc()

    # Token permutation per half: the program treats EVEN 128-token tiles
    # as key tiles.  For h=1 cores we swap each adjacent tile pair so THEIR
    # key tiles land on even positions.
    tok = np.arange(T)
    perm1 = 128 * ((tok // 128) ^ 1) + tok % 128  # swap adjacent 128-tiles

    xT = np.ascontiguousarray(x.transpose(0, 2, 1)).astype(BF)   # [B, C, T]
    xT1 = np.ascontiguousarray(xT[:, :, perm1])

    w_all = np.concatenate([Wq, Wk, Wv], axis=1).astype(np.float32)  # [C, 192]
    w_packed = np.ascontiguousarray(
        w_all.reshape(NC_, 128, 192).transpose(1, 0, 2).reshape(128, NC_ * 192)
    ).astype(BF)
    idn = np.eye(128, dtype=BF)

    # Masks for the diagonal pair: program key tile A holds global tile
    # 4st+h, tile B holds 4st+2+h; program query quarter g holds global
    # tile 4st+(g^h).  Causal test on global ids:
    #   A: 128h + k <= 128(g^h) + i   (cols 0:512, all four quarters)
    #   B: 128(2+h) + k <= 128(g^h) + i   (cols 512:768, quarters g=2,3)
    i = np.arange(128)[None, :]
    k = np.arange(128)[:, None]
    masks = {}
    for h in range(2):
        colsA = [(128 * h + k <= 128 * (g ^ h) + i) for g in range(4)]
        colsB = [(128 * (2 + h) + k <= 128 * (g ^ h) + i) for g in (2, 3)]
        masks[h] = np.concatenate(colsA + colsB, axis=1).astype(BF)  # [128,768]

    def pack_tiles(xTb):
        # [C, T] -> [128, tt*1024 + c*128 + t]
        return np.ascontiguousarray(
            xTb.reshape(NC_, 128, NTT, 128).transpose(1, 2, 0, 3)
            .reshape(128, NTT * 1024))

    in_maps = []
    for c in range(8):
        b, h = c % 4, c // 4
        xTb = xT[b] if h == 0 else xT1[b]
        in_maps.append({
            "xq": pack_tiles(xTb),
            "w": w_packed,
            "aux2": np.concatenate([masks[h], idn], axis=1),
        })

    res = bass_utils.run_bass_kernel_spmd(nc, in_maps, core_ids=list(range(8)),
                                          trace=_trace)
    _CACHE["last_results"] = res

    # Decode: U[c] [128, 8*260] -> [q_perm, 65]; un-permute h=1 tokens.
    O = np.empty((B, T, D), dtype=np.float32)
    for b in range(B):
        Uh = []
        for h in range(2):
            U = res.results[b + 4 * h]["out"]            # [128, 2080]
            U = U.reshape(128, NST, 4, 65).transpose(1, 2, 0, 3)
            U = U.reshape(T, 65)                          # permuted q order
            Uh.append(U[perm1] if h == 1 else U)          # global q order
        Ut = Uh[0] + Uh[1]
        O[b] = Ut[:, 0:64] / Ut[:, 64:65]
    return O


# revision 8
# speedup vs baseline: 1.0965x; 1.0005x over previous
"""Causal single-head attention (B=4, T=4096, C=1024, D=64) on 8 NeuronCores.

Sharding: core c = (batch b = c % 4, half h = c // 4).
Each core handles ALL queries of its batch against its half of the key
blocks (256-token blocks with block index ≡ h mod 2).  Pure SPMD: the
program is identical on every core; cores differ only in input data
(x[b]^T, block-pair-permuted for h=1, and the causal mask).  Each core
emits unnormalized U[q, 0:64] = sum_k exp(s) v and U[q, 64] = sum_k
exp(s); the host adds the two halves per batch and normalizes.

On-chip dataflow (bf16 compute, f32 PSUM accumulation):
  per 128-token tile tt:  PJ [128t, 192] = sum_c x_tile[c,tt]^T @ w[c]
    (fused Q|K|V projection, x tile stationary; non-key tiles: Q only)
  Q^T, K^T via PE transposes (deferred one unit for pipelining);
  V kept in [token, d] layout + ones column (softmax denominator)
  per query supertile st (512 q), local key tile pairs jp (diag first):
    S^T [128k, 2x512q] = K^T[j] @ Q^T[st]     (contraction over D=64)
    P^T = exp(S^T / 8)                        (one ACT instr per pair)
    diagonal pair: P^T *= mask                (DVE)
    U [128q, 65] += P^T-chunk^T @ [V_j | 1]   (P chunk stationary,
        deferred TWO pairs so the PE stays off the exp critical path)
  Projection units of the NEXT supertile interleave with attention
  pairs as fillers.  U -> SBUF -> DMA out per supertile (SWDGE/Pool
  path, keeping HWDGE free for input streaming).
"""
import sys
import numpy as np
import ml_dtypes
from collections import deque

if "/opt/trn_rl_repo" not in sys.path:
    sys.path.insert(0, "/opt/trn_rl_repo")

import concourse.bacc as bacc
import concourse.mybir as mybir
from concourse import tile
from concourse import bass_utils

bf16 = mybir.dt.bfloat16
f32 = mybir.dt.float32
BF = ml_dtypes.bfloat16

B, T, C, D = 4, 4096, 1024, 64
NC_ = C // 128      # 8 c-tiles
NTT = T // 128      # 32 token tiles
NST = 8             # query supertiles (512 q each)
STQ = 512

_CACHE = {}


def _build():
    nc = bacc.Bacc(None, target_bir_lowering=False, debug=False, num_devices=8)

    # xq tile-packed: xq[:, 1024*tt + 128*c : +128] = x^T[c-tile, token tile tt]
    xq = nc.dram_tensor("xq", [128, NTT * 1024], bf16, kind="ExternalInput")
    w = nc.dram_tensor("w", [128, NC_ * 192], bf16, kind="ExternalInput")
    # aux2 = diag mask [0:768] | identity [768:896]
    aux2 = nc.dram_tensor("aux2", [128, 896], bf16, kind="ExternalInput")
    out = nc.dram_tensor("out", [128, NST * 260], f32, kind="ExternalOutput")

    with tile.TileContext(nc) as tc:
        with tc.tile_pool(name="sb", bufs=1) as sb, \
             tc.tile_pool(name="qk", bufs=3) as qkp, \
             tc.tile_pool(name="pp", bufs=4) as pp, \
             tc.tile_pool(name="uo", bufs=2) as uop, \
             tc.tile_pool(name="ps", bufs=2, space="PSUM") as ps:

            # ---- resident tiles ----
            xq_sb = sb.tile([128, NTT * 1024], bf16, tag="xq")
            w_sb = sb.tile([128, NC_ * 192], bf16, tag="w")
            aux_sb = sb.tile([128, 896], bf16, tag="aux")
            msk_sb = aux_sb[:, 0:768]
            idn_sb = aux_sb[:, 768:896]
            qT = sb.tile([64, T], bf16, tag="qT")       # Q^T strip
            kT = sb.tile([64, T // 2], bf16, tag="kT")  # K^T local tiles
            v_sb = sb.tile([128, 16 * 65], bf16, tag="v")  # [V_j | 1] tiles

            # DMA order = consumption order: w, then one DMA per token tile
            # (each delivers ALL c-tiles of that tile, so projection unit tt
            # unblocks as soon as ITS chunk lands).
            nc.sync.dma_start(w_sb[:], w[:])
            for tt in range(NTT):
                nc.sync.dma_start(xq_sb[:, 1024 * tt:1024 * (tt + 1)],
                                  xq[:, 1024 * tt:1024 * (tt + 1)])
                if tt == 3:  # identity gates the first transposes (~chunk3)
                    nc.sync.dma_start(aux_sb[:], aux2[:])
            # ones columns of the V tiles (denominator trick)
            for j in range(16):
                nc.gpsimd.memset(v_sb[:, 65 * j + 64:65 * j + 65], 1.0)

            # PE p-state warmup: the PE ramps to full clock only after ~3us
            # of continuous busy; run junk matmuls on a zeroed tile right at
            # t=0 so the ramp completes before the first real projection.
            warm = sb.tile([128, 640], bf16, tag="warm")
            nc.vector.memset(warm[:], 0.0)
            # trigger the ACT Exp table load (1.28us) at t=0, not at the
            # first real exp
            nc.scalar.activation(warm[:, 0:1], warm[:, 0:1],
                                 mybir.ActivationFunctionType.Exp, scale=1.0)
            for i in range(5):
                wps = ps.tile([128, 512], f32, tag="s", name=f"warm{i}")
                nc.tensor.matmul(wps[:], warm[:, 0:128], warm[:, 128:640],
                                 start=True, stop=True)

            # ---- projection units, software-pipelined ----
            # mm-phase: fused projection matmuls into PSUM + DVE copies out.
            # tp-phase (transposes + Q^T/K^T copies) is deferred one unit so
            # the PE never waits on the just-issued DVE copy.
            proj_prev = [None]

            def xsrc(tt, c):
                return xq_sb[:, 1024 * tt + 128 * c:1024 * tt + 128 * (c + 1)]

            def proj_tp(state):
                tt, qk, tp = state
                is_key = (tt & 1) == 0
                if is_key:
                    j = tt // 2
                    nc.tensor.transpose(tp[:, 0:128], qk[:, 0:64], idn_sb)
                    nc.tensor.transpose(tp[:, 128:256], qk[:, 64:128], idn_sb)
                    nc.vector.tensor_copy(qT[:, 128 * tt:128 * (tt + 1)],
                                          tp[:, 0:128])
                    nc.vector.tensor_copy(kT[:, 128 * j:128 * (j + 1)],
                                          tp[:, 128:256])
                else:
                    nc.tensor.transpose(tp[:, 0:128], qk[:], idn_sb)
                    nc.vector.tensor_copy(qT[:, 128 * tt:128 * (tt + 1)],
                                          tp[:, 0:128])

            def proj_unit(tt):
                """Key-ness fixed to tt%2==0 (128-interleaved); h=1 cores get
                adjacent-tile-permuted inputs so their key tiles land on
                even positions."""
                is_key = (tt & 1) == 0
                if not is_key and tt % 2 == 1 and tt <= 19:
                    # The tile that gates attention(st): compute Q^T directly
                    # (w stationary), skipping the qk-copy + transpose hops,
                    # with the qT copy on ACT to dodge the DVE queue.
                    pjq = ps.tile([64, 128], f32, tag="pj", name=f"pjq{tt}")
                    for c in range(NC_):
                        nc.tensor.matmul(pjq[:], w_sb[:, 192 * c:192 * c + 64],
                                         xsrc(tt, c),
                                         start=(c == 0), stop=(c == NC_ - 1))
                    nc.scalar.activation(qT[:, 128 * tt:128 * (tt + 1)],
                                         pjq[:],
                                         mybir.ActivationFunctionType.Copy)
                    return
                wid = 192 if is_key else 64
                pj = ps.tile([128, wid], f32, tag="pj", name=f"pj{tt}")
                for c in range(NC_):
                    nc.tensor.matmul(pj[:], xsrc(tt, c),
                                     w_sb[:, 192 * c:192 * c + wid],
                                     start=(c == 0), stop=(c == NC_ - 1))
                qk = qkp.tile([128, 128 if is_key else 64], bf16, tag="qk",
                              name=f"qk{tt}")
                if is_key:
                    j = tt // 2
                    nc.vector.tensor_copy(qk[:], pj[:, 0:128])
                    nc.vector.tensor_copy(v_sb[:, 65 * j:65 * j + 64],
                                          pj[:, 128:192])
                else:
                    nc.vector.tensor_copy(qk[:], pj[:])
                tp = ps.tile([64, 256], bf16, tag="pj", name=f"tp{tt}")
                if proj_prev[0] is not None:
                    proj_tp(proj_prev[0])
                proj_prev[0] = (tt, qk, tp)

            def proj_flush():
                if proj_prev[0] is not None:
                    proj_tp(proj_prev[0])
                    proj_prev[0] = None

            # ---- attention: one continuous pair stream across supertiles ----
            # Global software pipeline: the U matmuls of a pair are deferred
            # two pairs (possibly crossing into the next supertile) so the
            # PE never sits on the ACT exp critical path, and the ACT stream
            # has no supertile-boundary bubble.
            u_state = {}   # st -> (u4 tile, n_emitted)
            pendings = []  # (st, jp, p2)

            def emit_u_d(st, jp, p2, d):
                if st not in u_state:
                    u_state[st] = [ps.tile([128, 260], f32, tag="u",
                                           name=f"u{st}", bufs=1), 0]
                ent = u_state[st]
                u4 = ent[0]
                j = 2 * jp + d
                total = 8 * (st + 1) - 2  # diag d1 contributes only g=2,3
                if jp == st and d == 1:
                    gs = [(2, 512), (3, 640)]
                else:
                    gs = [(g, 512 * d + 128 * g) for g in range(4)]
                for g, lo in gs:
                    # start=True zeroes the WHOLE PSUM bank: set it only
                    # on the chronologically first matmul into u4.
                    nc.tensor.matmul(
                        u4[:, 65 * g:65 * (g + 1)],
                        p2[:, lo:lo + 128],
                        v_sb[:, 65 * j:65 * (j + 1)],
                        start=(ent[1] == 0),
                        stop=(ent[1] == total - 1),
                        skip_group_check=True)
                    ent[1] += 1

            def ship(st):
                ent = u_state[st]
                if ent[1] == 8 * (st + 1) - 2:  # supertile complete -> ship
                    uo_t = uop.tile([128, 260], f32, tag="uo", name=f"uo{st}")
                    if st <= 2:
                        nc.scalar.activation(uo_t[:], ent[0][:],
                                             mybir.ActivationFunctionType.Copy)
                    else:
                        nc.vector.tensor_copy(uo_t[:], ent[0][:])
                    eng = nc.sync if st == NST - 1 else nc.gpsimd
                    eng.dma_start(out[:, 260 * st:260 * (st + 1)], uo_t[:])
                    del u_state[st]

            def emit_u(st, jp, p2):
                if jp == st:  # diagonal pair -> causal mask, deferred here so
                    # the DVE is free for critical copies at the boundary
                    nc.vector.tensor_mul(p2[:, 0:768], p2[:, 0:768], msk_sb)
                emit_u_d(st, jp, p2, 0)
                emit_u_d(st, jp, p2, 1)
                ship(st)

            def emit_pair(st, jp):
                qsl = slice(STQ * st, STQ * (st + 1))
                s2 = ps.tile([128, 1024], f32, tag="s", name=f"s{st}_{jp}")
                p2 = pp.tile([128, 1024], bf16, tag="p", name=f"p{st}_{jp}")
                if jp == st:
                    # diagonal pair: with 128-interleaved keys, the second
                    # tile is visible only to queries [256:512) for BOTH
                    # halves -> 768 live columns instead of 1024
                    nc.tensor.matmul(s2[:, 0:512],
                                     kT[:, 128 * 2 * jp:128 * (2 * jp + 1)],
                                     qT[:, qsl], start=True, stop=True)
                    nc.tensor.matmul(s2[:, 512:768],
                                     kT[:, 128 * (2 * jp + 1):128 * (2 * jp + 2)],
                                     qT[:, STQ * st + 256:STQ * (st + 1)],
                                     start=True, stop=True)
                    nc.scalar.activation(p2[:, 0:768], s2[:, 0:768],
                                         mybir.ActivationFunctionType.Exp,
                                         scale=0.125)
                else:
                    for d in range(2):
                        j = 2 * jp + d
                        nc.tensor.matmul(s2[:, 512 * d:512 * (d + 1)],
                                         kT[:, 128 * j:128 * (j + 1)],
                                         qT[:, qsl], start=True, stop=True)
                    nc.scalar.activation(p2[:], s2[:],
                                         mybir.ActivationFunctionType.Exp,
                                         scale=0.125)
                pendings.append((st, jp, p2))
                if len(pendings) > 3:
                    emit_u(*pendings.pop(0))

            def filler_q(qtr, half):
                return deque(
                    (lambda t: (lambda: proj_unit(t)))(8 * qtr + 4 * half + i)
                    for i in range(4))

            # Fillers (projection units of supertile st+1) are emitted inside
            # supertile st's pair loop, starting at the pair index where their
            # DMA chunk has arrived (the stream is DMA-paced early on).
            for tl in range(4):
                proj_unit(tl)
            schedule = [filler_q(0, 1), filler_q(1, 0), filler_q(1, 1),
                        filler_q(2, 0), filler_q(2, 1), filler_q(3, 0),
                        filler_q(3, 1), deque()]
            for st in range(8):
                fillers = schedule[st]
                proj_flush()  # this supertile's Q^T/K^T must be complete
                for pi, jp in enumerate([st] + list(range(st))):  # diag first
                    emit_pair(st, jp)
                    if fillers:
                        fillers.popleft()()
                while fillers:
                    fillers.popleft()()
            for pd in pendings:
                emit_u(*pd)
            pendings.clear()

    nc.compile()
    return nc


def _get_nc():
    if "nc" not in _CACHE:
        _CACHE["nc"] = _build()
    return _CACHE["nc"]


def kernel(x, Wq, Wk, Wv, _trace=False):
    x = np.asarray(x)
    nc = _get_nc()

    # Token permutation per half: the program treats EVEN 128-token tiles
    # as key tiles.  For h=1 cores we swap each adjacent tile pair so THEIR
    # key tiles land on even positions.
    tok = np.arange(T)
    perm1 = 128 * ((tok // 128) ^ 1) + tok % 128  # swap adjacent 128-tiles

    xT = np.ascontiguousarray(x.transpose(0, 2, 1)).astype(BF)   # [B, C, T]
    xT1 = np.ascontiguousarray(xT[:, :, perm1])

    w_all = np.concatenate([Wq, Wk, Wv], axis=1).astype(np.float32)  # [C, 192]
    w_packed = np.ascontiguousarray(
        w_all.reshape(NC_, 128, 192).transpose(1, 0, 2).reshape(128, NC_ * 192)
    ).astype(BF)
    idn = np.eye(128, dtype=BF)

    # Masks for the diagonal pair: program key tile A holds global tile
    # 4st+h, tile B holds 4st+2+h; program query quarter g holds global
    # tile 4st+(g^h).  Causal test on global ids:
    #   A: 128h + k <= 128(g^h) + i   (cols 0:512, all four quarters)
    #   B: 128(2+h) + k <= 128(g^h) + i   (cols 512:768, quarters g=2,3)
    i = np.arange(128)[None, :]
    k = np.arange(128)[:, None]
    masks = {}
    for h in range(2):
        colsA = [(128 * h + k <= 128 * (g ^ h) + i) for g in range(4)]
        colsB = [(128 * (2 + h) + k <= 128 * (g ^ h) + i) for g in (2, 3)]
        masks[h] = np.concatenate(colsA + colsB, axis=1).astype(BF)  # [128,768]

    def pack_tiles(xTb):
        # [C, T] -> [128, tt*1024 + c*128 + t]
        return np.ascontiguousarray(
            xTb.reshape(NC_, 128, NTT, 128).transpose(1, 2, 0, 3)
            .reshape(128, NTT * 1024))

    in_maps = []
    for c in range(8):
        b, h = c % 4, c // 4
        xTb = xT[b] if h == 0 else xT1[b]
        in_maps.append({
            "xq": pack_tiles(xTb),
            "w": w_packed,
            "aux2": np.concatenate([masks[h], idn], axis=1),
        })

    res = bass_utils.run_bass_kernel_spmd(nc, in_maps, core_ids=list(range(8)),
                                          trace=_trace)
    _CACHE["last_results"] = res

    # Decode: U[c] [128, 8*260] -> [q_perm, 65]; un-permute h=1 tokens.
    O = np.empty((B, T, D), dtype=np.float32)
    for b in range(B):
        Uh = []
        for h in range(2):
            U = res.results[b + 4 * h]["out"]            # [128, 2080]
            U = U.reshape(128, NST, 4, 65).transpose(1, 2, 0, 3)
            U = U.reshape(T, 65)                          # permuted q order
            Uh.append(U[perm1] if h == 1 else U)          # global q order
        Ut = Uh[0] + Uh[1]
        O[b] = Ut[:, 0:64] / Ut[:, 64:65]
    return O


# revision 9
# speedup vs baseline: 1.1105x; 1.0127x over previous
"""Causal single-head attention (B=4, T=4096, C=1024, D=64) on 8 NeuronCores.

Sharding: core c = (batch b = c % 4, half h = c // 4).
Each core handles ALL queries of its batch against its half of the key
blocks (256-token blocks with block index ≡ h mod 2).  Pure SPMD: the
program is identical on every core; cores differ only in input data
(x[b]^T, block-pair-permuted for h=1, and the causal mask).  Each core
emits unnormalized U[q, 0:64] = sum_k exp(s) v and U[q, 64] = sum_k
exp(s); the host adds the two halves per batch and normalizes.

On-chip dataflow (bf16 compute, f32 PSUM accumulation):
  per 128-token tile tt:  PJ [128t, 192] = sum_c x_tile[c,tt]^T @ w[c]
    (fused Q|K|V projection, x tile stationary; non-key tiles: Q only)
  Q^T, K^T via PE transposes (deferred one unit for pipelining);
  V kept in [token, d] layout + ones column (softmax denominator)
  per query supertile st (512 q), local key tile pairs jp (diag first):
    S^T [128k, 2x512q] = K^T[j] @ Q^T[st]     (contraction over D=64)
    P^T = exp(S^T / 8)                        (one ACT instr per pair)
    diagonal pair: P^T *= mask                (DVE)
    U [128q, 65] += P^T-chunk^T @ [V_j | 1]   (P chunk stationary,
        deferred TWO pairs so the PE stays off the exp critical path)
  Projection units of the NEXT supertile interleave with attention
  pairs as fillers.  U -> SBUF -> DMA out per supertile (SWDGE/Pool
  path, keeping HWDGE free for input streaming).
"""
import sys
import numpy as np
import ml_dtypes
from collections import deque

if "/opt/trn_rl_repo" not in sys.path:
    sys.path.insert(0, "/opt/trn_rl_repo")

import concourse.bacc as bacc
import concourse.mybir as mybir
from concourse import tile
from concourse import bass_utils

bf16 = mybir.dt.bfloat16
f32 = mybir.dt.float32
BF = ml_dtypes.bfloat16

B, T, C, D = 4, 4096, 1024, 64
NC_ = C // 128      # 8 c-tiles
NTT = T // 128      # 32 token tiles
NST = 8             # query supertiles (512 q each)
STQ = 512

_CACHE = {}


def _build():
    nc = bacc.Bacc(None, target_bir_lowering=False, debug=False, num_devices=8)

    # xq tile-packed: xq[:, 1024*tt + 128*c : +128] = x^T[c-tile, token tile tt]
    xq = nc.dram_tensor("xq", [128, NTT * 1024], bf16, kind="ExternalInput")
    w = nc.dram_tensor("w", [128, NC_ * 192], bf16, kind="ExternalInput")
    # aux2 = diag mask [0:768] | identity [768:896]
    aux2 = nc.dram_tensor("aux2", [128, 896], bf16, kind="ExternalInput")
    out = nc.dram_tensor("out", [128, NST * 260], f32, kind="ExternalOutput")

    with tile.TileContext(nc) as tc:
        with tc.tile_pool(name="sb", bufs=1) as sb, \
             tc.tile_pool(name="qk", bufs=3) as qkp, \
             tc.tile_pool(name="pp", bufs=5) as pp, \
             tc.tile_pool(name="uo", bufs=2) as uop, \
             tc.tile_pool(name="ps", bufs=2, space="PSUM") as ps:

            # ---- resident tiles ----
            xq_sb = sb.tile([128, NTT * 1024], bf16, tag="xq")
            w_sb = sb.tile([128, NC_ * 192], bf16, tag="w")
            aux_sb = sb.tile([128, 896], bf16, tag="aux")
            msk_sb = aux_sb[:, 0:768]
            idn_sb = aux_sb[:, 768:896]
            qT = sb.tile([64, T], bf16, tag="qT")       # Q^T strip
            kT = sb.tile([64, T // 2], bf16, tag="kT")  # K^T local tiles
            v_sb = sb.tile([128, 16 * 65], bf16, tag="v")  # [V_j | 1] tiles

            # DMA order = consumption order: w, then one DMA per token tile
            # (each delivers ALL c-tiles of that tile, so projection unit tt
            # unblocks as soon as ITS chunk lands).
            nc.sync.dma_start(w_sb[:], w[:])
            for tt in range(NTT):
                nc.sync.dma_start(xq_sb[:, 1024 * tt:1024 * (tt + 1)],
                                  xq[:, 1024 * tt:1024 * (tt + 1)])
                if tt == 3:  # identity gates the first transposes (~chunk3)
                    nc.sync.dma_start(aux_sb[:], aux2[:])
            # ones columns of the V tiles (denominator trick)
            for j in range(16):
                nc.gpsimd.memset(v_sb[:, 65 * j + 64:65 * j + 65], 1.0)

            # PE p-state warmup: the PE ramps to full clock only after ~3us
            # of continuous busy; run junk matmuls on a zeroed tile right at
            # t=0 so the ramp completes before the first real projection.
            warm = sb.tile([128, 640], bf16, tag="warm")
            nc.vector.memset(warm[:], 0.0)
            # trigger the ACT Exp table load (1.28us) at t=0, not at the
            # first real exp
            nc.scalar.activation(warm[:, 0:1], warm[:, 0:1],
                                 mybir.ActivationFunctionType.Exp, scale=1.0)
            for i in range(5):
                wps = ps.tile([128, 512], f32, tag="s", name=f"warm{i}")
                nc.tensor.matmul(wps[:], warm[:, 0:128], warm[:, 128:640],
                                 start=True, stop=True)

            # ---- projection units, software-pipelined ----
            # mm-phase: fused projection matmuls into PSUM + DVE copies out.
            # tp-phase (transposes + Q^T/K^T copies) is deferred one unit so
            # the PE never waits on the just-issued DVE copy.
            proj_prev = [None]

            def xsrc(tt, c):
                return xq_sb[:, 1024 * tt + 128 * c:1024 * tt + 128 * (c + 1)]

            def proj_tp(state):
                tt, qk, tp = state
                is_key = (tt & 1) == 0
                if is_key:
                    j = tt // 2
                    nc.tensor.transpose(tp[:, 0:128], qk[:, 0:64], idn_sb)
                    nc.tensor.transpose(tp[:, 128:256], qk[:, 64:128], idn_sb)
                    nc.vector.tensor_copy(qT[:, 128 * tt:128 * (tt + 1)],
                                          tp[:, 0:128])
                    nc.vector.tensor_copy(kT[:, 128 * j:128 * (j + 1)],
                                          tp[:, 128:256])
                else:
                    nc.tensor.transpose(tp[:, 0:128], qk[:], idn_sb)
                    nc.vector.tensor_copy(qT[:, 128 * tt:128 * (tt + 1)],
                                          tp[:, 0:128])

            def proj_unit(tt):
                """Key-ness fixed to tt%2==0 (128-interleaved); h=1 cores get
                adjacent-tile-permuted inputs so their key tiles land on
                even positions."""
                is_key = (tt & 1) == 0
                if not is_key and tt % 2 == 1 and tt <= 19:
                    # The tile that gates attention(st): compute Q^T directly
                    # (w stationary), skipping the qk-copy + transpose hops,
                    # with the qT copy on ACT to dodge the DVE queue.
                    pjq = ps.tile([64, 128], f32, tag="pj", name=f"pjq{tt}")
                    for c in range(NC_):
                        nc.tensor.matmul(pjq[:], w_sb[:, 192 * c:192 * c + 64],
                                         xsrc(tt, c),
                                         start=(c == 0), stop=(c == NC_ - 1))
                    nc.vector.tensor_copy(qT[:, 128 * tt:128 * (tt + 1)],
                                          pjq[:])
                    return
                wid = 192 if is_key else 64
                pj = ps.tile([128, wid], f32, tag="pj", name=f"pj{tt}")
                for c in range(NC_):
                    nc.tensor.matmul(pj[:], xsrc(tt, c),
                                     w_sb[:, 192 * c:192 * c + wid],
                                     start=(c == 0), stop=(c == NC_ - 1))
                qk = qkp.tile([128, 128 if is_key else 64], bf16, tag="qk",
                              name=f"qk{tt}")
                if is_key:
                    j = tt // 2
                    nc.vector.tensor_copy(qk[:], pj[:, 0:128])
                    nc.vector.tensor_copy(v_sb[:, 65 * j:65 * j + 64],
                                          pj[:, 128:192])
                else:
                    nc.vector.tensor_copy(qk[:], pj[:])
                tp = ps.tile([64, 256], bf16, tag="pj", name=f"tp{tt}")
                if proj_prev[0] is not None:
                    proj_tp(proj_prev[0])
                proj_prev[0] = (tt, qk, tp)

            def proj_flush():
                if proj_prev[0] is not None:
                    proj_tp(proj_prev[0])
                    proj_prev[0] = None

            # ---- attention: one continuous pair stream across supertiles ----
            # Global software pipeline: the U matmuls of a pair are deferred
            # two pairs (possibly crossing into the next supertile) so the
            # PE never sits on the ACT exp critical path, and the ACT stream
            # has no supertile-boundary bubble.
            u_state = {}   # st -> (u4 tile, n_emitted)
            pendings = []  # (st, jp, p2)

            def emit_u_d(st, jp, p2, d):
                if st not in u_state:
                    u_state[st] = [ps.tile([128, 260], f32, tag="u",
                                           name=f"u{st}", bufs=1), 0]
                ent = u_state[st]
                u4 = ent[0]
                j = 2 * jp + d
                total = 8 * (st + 1) - 2  # diag d1 contributes only g=2,3
                if jp == st and d == 1:
                    gs = [(2, 512), (3, 640)]
                else:
                    gs = [(g, 512 * d + 128 * g) for g in range(4)]
                for g, lo in gs:
                    # start=True zeroes the WHOLE PSUM bank: set it only
                    # on the chronologically first matmul into u4.
                    nc.tensor.matmul(
                        u4[:, 65 * g:65 * (g + 1)],
                        p2[:, lo:lo + 128],
                        v_sb[:, 65 * j:65 * (j + 1)],
                        start=(ent[1] == 0),
                        stop=(ent[1] == total - 1),
                        skip_group_check=True)
                    ent[1] += 1

            def ship(st):
                ent = u_state[st]
                if ent[1] == 8 * (st + 1) - 2:  # supertile complete -> ship
                    uo_t = uop.tile([128, 260], f32, tag="uo", name=f"uo{st}")
                    if st <= 2:
                        nc.scalar.activation(uo_t[:], ent[0][:],
                                             mybir.ActivationFunctionType.Copy)
                    else:
                        nc.vector.tensor_copy(uo_t[:], ent[0][:])
                    eng = nc.sync if st == NST - 1 else nc.gpsimd
                    eng.dma_start(out[:, 260 * st:260 * (st + 1)], uo_t[:])
                    del u_state[st]

            def emit_u(st, jp, p2):
                if jp == st:  # diagonal pair -> causal mask, deferred here so
                    # the DVE is free for critical copies at the boundary
                    nc.vector.tensor_mul(p2[:, 0:768], p2[:, 0:768], msk_sb)
                emit_u_d(st, jp, p2, 0)
                emit_u_d(st, jp, p2, 1)
                ship(st)

            def emit_pair(st, jp):
                qsl = slice(STQ * st, STQ * (st + 1))
                s2 = ps.tile([128, 1024], f32, tag="s", name=f"s{st}_{jp}")
                p2 = pp.tile([128, 1024], bf16, tag="p", name=f"p{st}_{jp}")
                if jp == st:
                    # diagonal pair: with 128-interleaved keys, the second
                    # tile is visible only to queries [256:512) for BOTH
                    # halves -> 768 live columns instead of 1024
                    nc.tensor.matmul(s2[:, 0:512],
                                     kT[:, 128 * 2 * jp:128 * (2 * jp + 1)],
                                     qT[:, qsl], start=True, stop=True)
                    nc.tensor.matmul(s2[:, 512:768],
                                     kT[:, 128 * (2 * jp + 1):128 * (2 * jp + 2)],
                                     qT[:, STQ * st + 256:STQ * (st + 1)],
                                     start=True, stop=True)
                    nc.scalar.activation(p2[:, 0:768], s2[:, 0:768],
                                         mybir.ActivationFunctionType.Exp,
                                         scale=0.125)
                else:
                    for d in range(2):
                        j = 2 * jp + d
                        nc.tensor.matmul(s2[:, 512 * d:512 * (d + 1)],
                                         kT[:, 128 * j:128 * (j + 1)],
                                         qT[:, qsl], start=True, stop=True)
                    nc.scalar.activation(p2[:], s2[:],
                                         mybir.ActivationFunctionType.Exp,
                                         scale=0.125)
                pendings.append((st, jp, p2))
                if len(pendings) > 4:
                    emit_u(*pendings.pop(0))

            def filler_q(qtr, half):
                return deque(
                    (lambda t: (lambda: proj_unit(t)))(8 * qtr + 4 * half + i)
                    for i in range(4))

            # Fillers (projection units of supertile st+1) are emitted inside
            # supertile st's pair loop, starting at the pair index where their
            # DMA chunk has arrived (the stream is DMA-paced early on).
            for tl in range(4):
                proj_unit(tl)
            schedule = [filler_q(0, 1), filler_q(1, 0), filler_q(1, 1),
                        filler_q(2, 0), filler_q(2, 1), filler_q(3, 0),
                        filler_q(3, 1), deque()]
            for st in range(8):
                fillers = schedule[st]
                proj_flush()  # this supertile's Q^T/K^T must be complete
                for pi, jp in enumerate([st] + list(range(st))):  # diag first
                    emit_pair(st, jp)
                    if fillers:
                        fillers.popleft()()
                while fillers:
                    fillers.popleft()()
            for pd in pendings:
                emit_u(*pd)
            pendings.clear()

    nc.compile()
    return nc


def _get_nc():
    if "nc" not in _CACHE:
        _CACHE["nc"] = _build()
    return _CACHE["nc"]


def kernel(x, Wq, Wk, Wv, _trace=False):
    x = np.asarray(x)
    nc = _get_nc()

    # Token permutation per half: the program treats EVEN 128-token tiles
    # as key tiles.  For h=1 cores we swap each adjacent tile pair so THEIR
    # key tiles land on even positions.
    tok = np.arange(T)
    perm1 = 128 * ((tok // 128) ^ 1) + tok % 128  # swap adjacent 128-tiles

    xT = np.ascontiguousarray(x.transpose(0, 2, 1)).astype(BF)   # [B, C, T]
    xT1 = np.ascontiguousarray(xT[:, :, perm1])

    w_all = np.concatenate([Wq, Wk, Wv], axis=1).astype(np.float32)  # [C, 192]
    w_packed = np.ascontiguousarray(
        w_all.reshape(NC_, 128, 192).transpose(1, 0, 2).reshape(128, NC_ * 192)
    ).astype(BF)
    idn = np.eye(128, dtype=BF)

    # Masks for the diagonal pair: program key tile A holds global tile
    # 4st+h, tile B holds 4st+2+h; program query quarter g holds global
    # tile 4st+(g^h).  Causal test on global ids:
    #   A: 128h + k <= 128(g^h) + i   (cols 0:512, all four quarters)
    #   B: 128(2+h) + k <= 128(g^h) + i   (cols 512:768, quarters g=2,3)
    i = np.arange(128)[None, :]
    k = np.arange(128)[:, None]
    masks = {}
    for h in range(2):
        colsA = [(128 * h + k <= 128 * (g ^ h) + i) for g in range(4)]
        colsB = [(128 * (2 + h) + k <= 128 * (g ^ h) + i) for g in (2, 3)]
        masks[h] = np.concatenate(colsA + colsB, axis=1).astype(BF)  # [128,768]

    def pack_tiles(xTb):
        # [C, T] -> [128, tt*1024 + c*128 + t]
        return np.ascontiguousarray(
            xTb.reshape(NC_, 128, NTT, 128).transpose(1, 2, 0, 3)
            .reshape(128, NTT * 1024))

    in_maps = []
    for c in range(8):
        b, h = c % 4, c // 4
        xTb = xT[b] if h == 0 else xT1[b]
        in_maps.append({
            "xq": pack_tiles(xTb),
            "w": w_packed,
            "aux2": np.concatenate([masks[h], idn], axis=1),
        })

    res = bass_utils.run_bass_kernel_spmd(nc, in_maps, core_ids=list(range(8)),
                                          trace=_trace)
    _CACHE["last_results"] = res

    # Decode: U[c] [128, 8*260] -> [q_perm, 65]; un-permute h=1 tokens.
    O = np.empty((B, T, D), dtype=np.float32)
    for b in range(B):
        Uh = []
        for h in range(2):
            U = res.results[b + 4 * h]["out"]            # [128, 2080]
            U = U.reshape(128, NST, 4, 65).transpose(1, 2, 0, 3)
            U = U.reshape(T, 65)                          # permuted q order
            Uh.append(U[perm1] if h == 1 else U)          # global q order
        Ut = Uh[0] + Uh[1]
        O[b] = Ut[:, 0:64] / Ut[:, 64:65]
    return O


# revision 10
# speedup vs baseline: 1.1767x; 1.0596x over previous
"""Causal single-head attention (B=4, T=4096, C=1024, D=64) on 8 NeuronCores.

Sharding: core c = (batch b = c % 4, half h = c // 4).
Each core handles ALL queries of its batch against its half of the key
blocks (256-token blocks with block index ≡ h mod 2).  Pure SPMD: the
program is identical on every core; cores differ only in input data
(x[b]^T, block-pair-permuted for h=1, and the causal mask).  Each core
emits unnormalized U[q, 0:64] = sum_k exp(s) v and U[q, 64] = sum_k
exp(s); the host adds the two halves per batch and normalizes.

On-chip dataflow (bf16 compute, f32 PSUM accumulation):
  per 128-token tile tt:  PJ [128t, 192] = sum_c x_tile[c,tt]^T @ w[c]
    (fused Q|K|V projection, x tile stationary; non-key tiles: Q only)
  Q^T, K^T via PE transposes (deferred one unit for pipelining);
  V kept in [token, d] layout + ones column (softmax denominator)
  per query supertile st (512 q), local key tile pairs jp (diag first):
    S^T [128k, 2x512q] = K^T[j] @ Q^T[st]     (contraction over D=64)
    P^T = exp(S^T / 8)                        (one ACT instr per pair)
    diagonal pair: P^T *= mask                (DVE)
    U [128q, 65] += P^T-chunk^T @ [V_j | 1]   (P chunk stationary,
        deferred TWO pairs so the PE stays off the exp critical path)
  Projection units of the NEXT supertile interleave with attention
  pairs as fillers.  U -> SBUF -> DMA out per supertile (SWDGE/Pool
  path, keeping HWDGE free for input streaming).
"""
import sys
import numpy as np
import ml_dtypes
from collections import deque

if "/opt/trn_rl_repo" not in sys.path:
    sys.path.insert(0, "/opt/trn_rl_repo")

import concourse.bacc as bacc
import concourse.mybir as mybir
from concourse import tile
from concourse import bass_utils

bf16 = mybir.dt.bfloat16
f32 = mybir.dt.float32
BF = ml_dtypes.bfloat16

B, T, C, D = 4, 4096, 1024, 64
NC_ = C // 128      # 8 c-tiles
NTT = T // 128      # 32 token tiles
NST = 8             # query supertiles (512 q each)
STQ = 512

_CACHE = {}


def _build():
    nc = bacc.Bacc(None, target_bir_lowering=False, debug=False, num_devices=8)

    # xq tile-packed: xq[:, 1024*tt + 128*c : +128] = x^T[c-tile, token tile tt]
    xq = nc.dram_tensor("xq", [128, NTT * 1024], bf16, kind="ExternalInput")
    w = nc.dram_tensor("w", [128, NC_ * 192], bf16, kind="ExternalInput")
    # aux2 = diag mask [0:768] | identity [768:896]
    aux2 = nc.dram_tensor("aux2", [128, 896], bf16, kind="ExternalInput")
    out = nc.dram_tensor("out", [128, NST * 260], f32, kind="ExternalOutput")

    with tile.TileContext(nc) as tc:
        with tc.tile_pool(name="sb", bufs=1) as sb, \
             tc.tile_pool(name="qk", bufs=3) as qkp, \
             tc.tile_pool(name="pp", bufs=6) as pp, \
             tc.tile_pool(name="uo", bufs=2) as uop, \
             tc.tile_pool(name="ps", bufs=2, space="PSUM") as ps:

            # ---- resident tiles ----
            xq_sb = sb.tile([128, NTT * 1024], bf16, tag="xq")
            w_sb = sb.tile([128, NC_ * 192], bf16, tag="w")
            aux_sb = sb.tile([128, 896], bf16, tag="aux")
            msk_sb = aux_sb[:, 0:768]
            idn_sb = aux_sb[:, 768:896]
            qT = sb.tile([64, T], bf16, tag="qT")       # Q^T strip
            kT = sb.tile([64, T // 2], bf16, tag="kT")  # K^T local tiles
            v_sb = sb.tile([128, 16 * 65], bf16, tag="v")  # [V_j | 1] tiles

            # DMA order = consumption order: w, then one DMA per token tile
            # (each delivers ALL c-tiles of that tile, so projection unit tt
            # unblocks as soon as ITS chunk lands).
            nc.sync.dma_start(w_sb[:], w[:])
            for tt in range(NTT):
                nc.sync.dma_start(xq_sb[:, 1024 * tt:1024 * (tt + 1)],
                                  xq[:, 1024 * tt:1024 * (tt + 1)])
                if tt == 3:  # identity gates the first transposes (~chunk3)
                    nc.sync.dma_start(aux_sb[:], aux2[:])
            # ones columns of the V tiles (denominator trick)
            for j in range(16):
                nc.gpsimd.memset(v_sb[:, 65 * j + 64:65 * j + 65], 1.0)

            # PE p-state warmup: the PE ramps to full clock only after ~3us
            # of continuous busy; run junk matmuls on a zeroed tile right at
            # t=0 so the ramp completes before the first real projection.
            warm = sb.tile([128, 640], bf16, tag="warm")
            nc.vector.memset(warm[:], 0.0)
            # trigger the ACT Exp table load (1.28us) at t=0, not at the
            # first real exp
            nc.scalar.activation(warm[:, 0:1], warm[:, 0:1],
                                 mybir.ActivationFunctionType.Exp, scale=1.0)
            for i in range(5):
                wps = ps.tile([128, 512], f32, tag="s", name=f"warm{i}")
                nc.tensor.matmul(wps[:], warm[:, 0:128], warm[:, 128:640],
                                 start=True, stop=True)

            # ---- projection units, software-pipelined ----
            # mm-phase: fused projection matmuls into PSUM + DVE copies out.
            # tp-phase (transposes + Q^T/K^T copies) is deferred one unit so
            # the PE never waits on the just-issued DVE copy.
            proj_prev = [None]

            def xsrc(tt, c):
                return xq_sb[:, 1024 * tt + 128 * c:1024 * tt + 128 * (c + 1)]

            def proj_tp(state):
                tt, qk, tp = state
                is_key = (tt & 1) == 0
                if is_key:
                    j = tt // 2
                    nc.tensor.transpose(tp[:, 0:128], qk[:, 0:64], idn_sb)
                    nc.tensor.transpose(tp[:, 128:256], qk[:, 64:128], idn_sb)
                    nc.vector.tensor_copy(qT[:, 128 * tt:128 * (tt + 1)],
                                          tp[:, 0:128])
                    nc.vector.tensor_copy(kT[:, 128 * j:128 * (j + 1)],
                                          tp[:, 128:256])
                else:
                    nc.tensor.transpose(tp[:, 0:128], qk[:], idn_sb)
                    nc.vector.tensor_copy(qT[:, 128 * tt:128 * (tt + 1)],
                                          tp[:, 0:128])

            def proj_unit(tt):
                """Key-ness fixed to tt%2==0 (128-interleaved); h=1 cores get
                adjacent-tile-permuted inputs so their key tiles land on
                even positions."""
                is_key = (tt & 1) == 0
                if not is_key and tt % 2 == 1 and tt <= 19:
                    # The tile that gates attention(st): compute Q^T directly
                    # (w stationary), skipping the qk-copy + transpose hops,
                    # with the qT copy on ACT to dodge the DVE queue.
                    pjq = ps.tile([64, 128], f32, tag="pj", name=f"pjq{tt}", bufs=3)
                    for c in range(NC_):
                        nc.tensor.matmul(pjq[:], w_sb[:, 192 * c:192 * c + 64],
                                         xsrc(tt, c),
                                         start=(c == 0), stop=(c == NC_ - 1))
                    nc.vector.tensor_copy(qT[:, 128 * tt:128 * (tt + 1)],
                                          pjq[:])
                    return
                wid = 192 if is_key else 64
                pj = ps.tile([128, wid], f32, tag="pj", name=f"pj{tt}", bufs=3)
                for c in range(NC_):
                    nc.tensor.matmul(pj[:], xsrc(tt, c),
                                     w_sb[:, 192 * c:192 * c + wid],
                                     start=(c == 0), stop=(c == NC_ - 1))
                qk = qkp.tile([128, 128 if is_key else 64], bf16, tag="qk",
                              name=f"qk{tt}")
                if is_key:
                    j = tt // 2
                    nc.vector.tensor_copy(qk[:], pj[:, 0:128])
                    nc.vector.tensor_copy(v_sb[:, 65 * j:65 * j + 64],
                                          pj[:, 128:192])
                else:
                    nc.vector.tensor_copy(qk[:], pj[:])
                tp = ps.tile([64, 256], bf16, tag="pj", name=f"tp{tt}", bufs=3)
                if proj_prev[0] is not None:
                    proj_tp(proj_prev[0])
                proj_prev[0] = (tt, qk, tp)

            def proj_flush():
                if proj_prev[0] is not None:
                    proj_tp(proj_prev[0])
                    proj_prev[0] = None

            # ---- attention: one continuous pair stream across supertiles ----
            # Global software pipeline: the U matmuls of a pair are deferred
            # two pairs (possibly crossing into the next supertile) so the
            # PE never sits on the ACT exp critical path, and the ACT stream
            # has no supertile-boundary bubble.
            u_state = {}   # st -> (u4 tile, n_emitted)
            pendings = []  # (st, jp, p2)

            def emit_u_d(st, jp, p2, d):
                if st not in u_state:
                    u_state[st] = [ps.tile([128, 260], f32, tag="u",
                                           name=f"u{st}", bufs=1), 0]
                ent = u_state[st]
                u4 = ent[0]
                j = 2 * jp + d
                total = 8 * (st + 1) - 2  # diag d1 contributes only g=2,3
                if jp == st and d == 1:
                    gs = [(2, 512), (3, 640)]
                else:
                    gs = [(g, 512 * d + 128 * g) for g in range(4)]
                for g, lo in gs:
                    # start=True zeroes the WHOLE PSUM bank: set it only
                    # on the chronologically first matmul into u4.
                    nc.tensor.matmul(
                        u4[:, 65 * g:65 * (g + 1)],
                        p2[:, lo:lo + 128],
                        v_sb[:, 65 * j:65 * (j + 1)],
                        start=(ent[1] == 0),
                        stop=(ent[1] == total - 1),
                        skip_group_check=True)
                    ent[1] += 1

            def ship(st):
                ent = u_state[st]
                if ent[1] == 8 * (st + 1) - 2:  # supertile complete -> ship
                    uo_t = uop.tile([128, 260], f32, tag="uo", name=f"uo{st}")
                    if st <= 2:
                        nc.scalar.activation(uo_t[:], ent[0][:],
                                             mybir.ActivationFunctionType.Copy)
                    else:
                        nc.vector.tensor_copy(uo_t[:], ent[0][:])
                    eng = nc.sync if st == NST - 1 else nc.gpsimd
                    eng.dma_start(out[:, 260 * st:260 * (st + 1)], uo_t[:])
                    del u_state[st]

            def emit_u(st, jp, p2):
                if jp == st:  # diagonal pair -> causal mask, deferred here so
                    # the DVE is free for critical copies at the boundary
                    nc.vector.tensor_mul(p2[:, 0:768], p2[:, 0:768], msk_sb)
                emit_u_d(st, jp, p2, 0)
                emit_u_d(st, jp, p2, 1)
                ship(st)

            def emit_pair(st, jp):
                qsl = slice(STQ * st, STQ * (st + 1))
                s2 = ps.tile([128, 1024], f32, tag="s", name=f"s{st}_{jp}")
                p2 = pp.tile([128, 1024], bf16, tag="p", name=f"p{st}_{jp}")
                if jp == st:
                    # diagonal pair: with 128-interleaved keys, the second
                    # tile is visible only to queries [256:512) for BOTH
                    # halves -> 768 live columns instead of 1024
                    nc.tensor.matmul(s2[:, 0:512],
                                     kT[:, 128 * 2 * jp:128 * (2 * jp + 1)],
                                     qT[:, qsl], start=True, stop=True)
                    nc.tensor.matmul(s2[:, 512:768],
                                     kT[:, 128 * (2 * jp + 1):128 * (2 * jp + 2)],
                                     qT[:, STQ * st + 256:STQ * (st + 1)],
                                     start=True, stop=True)
                    nc.scalar.activation(p2[:, 0:768], s2[:, 0:768],
                                         mybir.ActivationFunctionType.Exp,
                                         scale=0.125)
                else:
                    for d in range(2):
                        j = 2 * jp + d
                        nc.tensor.matmul(s2[:, 512 * d:512 * (d + 1)],
                                         kT[:, 128 * j:128 * (j + 1)],
                                         qT[:, qsl], start=True, stop=True)
                    nc.scalar.activation(p2[:], s2[:],
                                         mybir.ActivationFunctionType.Exp,
                                         scale=0.125)
                pendings.append((st, jp, p2))
                if len(pendings) > 5:
                    emit_u(*pendings.pop(0))

            def filler_q(qtr, half):
                return deque(
                    (lambda t: (lambda: proj_unit(t)))(8 * qtr + 4 * half + i)
                    for i in range(4))

            # Fillers (projection units of supertile st+1) are emitted inside
            # supertile st's pair loop, starting at the pair index where their
            # DMA chunk has arrived (the stream is DMA-paced early on).
            for tl in range(4):
                proj_unit(tl)
            schedule = [filler_q(0, 1), filler_q(1, 0), filler_q(1, 1),
                        filler_q(2, 0), filler_q(2, 1), filler_q(3, 0),
                        filler_q(3, 1), deque()]
            for st in range(8):
                fillers = schedule[st]
                proj_flush()  # this supertile's Q^T/K^T must be complete
                for pi, jp in enumerate([st] + list(range(st))):  # diag first
                    emit_pair(st, jp)
                    if fillers:
                        fillers.popleft()()
                while fillers:
                    fillers.popleft()()
            for pd in pendings:
                emit_u(*pd)
            pendings.clear()

    nc.compile()
    return nc


def _get_nc():
    if "nc" not in _CACHE:
        _CACHE["nc"] = _build()
    return _CACHE["nc"]


def kernel(x, Wq, Wk, Wv, _trace=False):
    x = np.asarray(x)
    nc = _get_nc()

    # Token permutation per half: the program treats EVEN 128-token tiles
    # as key tiles.  For h=1 cores we swap each adjacent tile pair so THEIR
    # key tiles land on even positions.
    tok = np.arange(T)
    perm1 = 128 * ((tok // 128) ^ 1) + tok % 128  # swap adjacent 128-tiles

    xT = np.ascontiguousarray(x.transpose(0, 2, 1)).astype(BF)   # [B, C, T]
    xT1 = np.ascontiguousarray(xT[:, :, perm1])

    w_all = np.concatenate([Wq, Wk, Wv], axis=1).astype(np.float32)  # [C, 192]
    w_packed = np.ascontiguousarray(
        w_all.reshape(NC_, 128, 192).transpose(1, 0, 2).reshape(128, NC_ * 192)
    ).astype(BF)
    idn = np.eye(128, dtype=BF)

    # Masks for the diagonal pair: program key tile A holds global tile
    # 4st+h, tile B holds 4st+2+h; program query quarter g holds global
    # tile 4st+(g^h).  Causal test on global ids:
    #   A: 128h + k <= 128(g^h) + i   (cols 0:512, all four quarters)
    #   B: 128(2+h) + k <= 128(g^h) + i   (cols 512:768, quarters g=2,3)
    i = np.arange(128)[None, :]
    k = np.arange(128)[:, None]
    masks = {}
    for h in range(2):
        colsA = [(128 * h + k <= 128 * (g ^ h) + i) for g in range(4)]
        colsB = [(128 * (2 + h) + k <= 128 * (g ^ h) + i) for g in (2, 3)]
        masks[h] = np.concatenate(colsA + colsB, axis=1).astype(BF)  # [128,768]

    def pack_tiles(xTb):
        # [C, T] -> [128, tt*1024 + c*128 + t]
        return np.ascontiguousarray(
            xTb.reshape(NC_, 128, NTT, 128).transpose(1, 2, 0, 3)
            .reshape(128, NTT * 1024))

    in_maps = []
    for c in range(8):
        b, h = c % 4, c // 4
        xTb = xT[b] if h == 0 else xT1[b]
        in_maps.append({
            "xq": pack_tiles(xTb),
            "w": w_packed,
            "aux2": np.concatenate([masks[h], idn], axis=1),
        })

    res = bass_utils.run_bass_kernel_spmd(nc, in_maps, core_ids=list(range(8)),
                                          trace=_trace)
    _CACHE["last_results"] = res

    # Decode: U[c] [128, 8*260] -> [q_perm, 65]; un-permute h=1 tokens.
    O = np.empty((B, T, D), dtype=np.float32)
    for b in range(B):
        Uh = []
        for h in range(2):
            U = res.results[b + 4 * h]["out"]            # [128, 2080]
            U = U.reshape(128, NST, 4, 65).transpose(1, 2, 0, 3)
            U = U.reshape(T, 65)                          # permuted q order
            Uh.append(U[perm1] if h == 1 else U)          # global q order
        Ut = Uh[0] + Uh[1]
        O[b] = Ut[:, 0:64] / Ut[:, 64:65]
    return O


# revision 11
# speedup vs baseline: 1.1797x; 1.0026x over previous
"""Causal single-head attention (B=4, T=4096, C=1024, D=64) on 8 NeuronCores.

Sharding: core c = (batch b = c % 4, half h = c // 4).
Each core handles ALL queries of its batch against its half of the key
blocks (256-token blocks with block index ≡ h mod 2).  Pure SPMD: the
program is identical on every core; cores differ only in input data
(x[b]^T, block-pair-permuted for h=1, and the causal mask).  Each core
emits unnormalized U[q, 0:64] = sum_k exp(s) v and U[q, 64] = sum_k
exp(s); the host adds the two halves per batch and normalizes.

On-chip dataflow (bf16 compute, f32 PSUM accumulation):
  per 128-token tile tt:  PJ [128t, 192] = sum_c x_tile[c,tt]^T @ w[c]
    (fused Q|K|V projection, x tile stationary; non-key tiles: Q only)
  Q^T, K^T via PE transposes (deferred one unit for pipelining);
  V kept in [token, d] layout + ones column (softmax denominator)
  per query supertile st (512 q), local key tile pairs jp (diag first):
    S^T [128k, 2x512q] = K^T[j] @ Q^T[st]     (contraction over D=64)
    P^T = exp(S^T / 8)                        (one ACT instr per pair)
    diagonal pair: P^T *= mask                (DVE)
    U [128q, 65] += P^T-chunk^T @ [V_j | 1]   (P chunk stationary,
        deferred TWO pairs so the PE stays off the exp critical path)
  Projection units of the NEXT supertile interleave with attention
  pairs as fillers.  U -> SBUF -> DMA out per supertile (SWDGE/Pool
  path, keeping HWDGE free for input streaming).
"""
import sys
import numpy as np
import ml_dtypes
from collections import deque

if "/opt/trn_rl_repo" not in sys.path:
    sys.path.insert(0, "/opt/trn_rl_repo")

import concourse.bacc as bacc
import concourse.mybir as mybir
from concourse import tile
from concourse import bass_utils

bf16 = mybir.dt.bfloat16
f32 = mybir.dt.float32
BF = ml_dtypes.bfloat16

B, T, C, D = 4, 4096, 1024, 64
NC_ = C // 128      # 8 c-tiles
NTT = T // 128      # 32 token tiles
NST = 8             # query supertiles (512 q each)
STQ = 512

_CACHE = {}


def _build():
    nc = bacc.Bacc(None, target_bir_lowering=False, debug=False, num_devices=8)

    # xq tile-packed: xq[:, 1024*tt + 128*c : +128] = x^T[c-tile, token tile tt]
    xq = nc.dram_tensor("xq", [128, NTT * 1024], bf16, kind="ExternalInput")
    w = nc.dram_tensor("w", [128, NC_ * 192], bf16, kind="ExternalInput")
    # aux2 = diag mask [0:768] | identity [768:896]
    aux2 = nc.dram_tensor("aux2", [128, 896], bf16, kind="ExternalInput")
    out = nc.dram_tensor("out", [128, NST * 260], f32, kind="ExternalOutput")

    with tile.TileContext(nc) as tc:
        with tc.tile_pool(name="sb", bufs=1) as sb, \
             tc.tile_pool(name="qk", bufs=3) as qkp, \
             tc.tile_pool(name="pp", bufs=6) as pp, \
             tc.tile_pool(name="uo", bufs=2) as uop, \
             tc.tile_pool(name="ps", bufs=2, space="PSUM") as ps:

            # ---- resident tiles ----
            xq_sb = sb.tile([128, NTT * 1024], bf16, tag="xq")
            w_sb = sb.tile([128, NC_ * 192], bf16, tag="w")
            aux_sb = sb.tile([128, 896], bf16, tag="aux")
            msk_sb = aux_sb[:, 0:768]
            idn_sb = aux_sb[:, 768:896]
            qT = sb.tile([64, T], bf16, tag="qT")       # Q^T strip
            kT = sb.tile([64, T // 2], bf16, tag="kT")  # K^T local tiles
            v_sb = sb.tile([128, 16 * 65], bf16, tag="v")  # [V_j | 1] tiles

            # DMA order = consumption order: w, then one DMA per token tile
            # (each delivers ALL c-tiles of that tile, so projection unit tt
            # unblocks as soon as ITS chunk lands).
            nc.sync.dma_start(w_sb[:], w[:])
            for tt in range(NTT):
                nc.sync.dma_start(xq_sb[:, 1024 * tt:1024 * (tt + 1)],
                                  xq[:, 1024 * tt:1024 * (tt + 1)])
                if tt == 2:  # identity gates the first transposes (~chunk3)
                    nc.sync.dma_start(aux_sb[:], aux2[:])
            # ones columns of the V tiles (denominator trick)
            for j in range(16):
                nc.gpsimd.memset(v_sb[:, 65 * j + 64:65 * j + 65], 1.0)

            # PE p-state warmup: the PE ramps to full clock only after ~3us
            # of continuous busy; run junk matmuls on a zeroed tile right at
            # t=0 so the ramp completes before the first real projection.
            warm = sb.tile([128, 640], bf16, tag="warm")
            nc.vector.memset(warm[:], 0.0)
            # trigger the ACT Exp table load (1.28us) at t=0, not at the
            # first real exp
            nc.scalar.activation(warm[:, 0:1], warm[:, 0:1],
                                 mybir.ActivationFunctionType.Exp, scale=1.0)
            for i in range(5):
                wps = ps.tile([128, 512], f32, tag="s", name=f"warm{i}")
                nc.tensor.matmul(wps[:], warm[:, 0:128], warm[:, 128:640],
                                 start=True, stop=True)

            # ---- projection units, software-pipelined ----
            # mm-phase: fused projection matmuls into PSUM + DVE copies out.
            # tp-phase (transposes + Q^T/K^T copies) is deferred one unit so
            # the PE never waits on the just-issued DVE copy.
            proj_prev = [None]

            def xsrc(tt, c):
                return xq_sb[:, 1024 * tt + 128 * c:1024 * tt + 128 * (c + 1)]

            def proj_tp(state):
                tt, qk, tp = state
                is_key = (tt & 1) == 0
                if is_key:
                    j = tt // 2
                    nc.tensor.transpose(tp[:, 0:128], qk[:, 0:64], idn_sb)
                    nc.tensor.transpose(tp[:, 128:256], qk[:, 64:128], idn_sb)
                    nc.vector.tensor_copy(qT[:, 128 * tt:128 * (tt + 1)],
                                          tp[:, 0:128])
                    nc.vector.tensor_copy(kT[:, 128 * j:128 * (j + 1)],
                                          tp[:, 128:256])
                else:
                    nc.tensor.transpose(tp[:, 0:128], qk[:], idn_sb)
                    nc.vector.tensor_copy(qT[:, 128 * tt:128 * (tt + 1)],
                                          tp[:, 0:128])

            def proj_unit(tt):
                """Key-ness fixed to tt%2==0 (128-interleaved); h=1 cores get
                adjacent-tile-permuted inputs so their key tiles land on
                even positions."""
                is_key = (tt & 1) == 0
                if not is_key and tt % 2 == 1 and tt <= 19:
                    # The tile that gates attention(st): compute Q^T directly
                    # (w stationary), skipping the qk-copy + transpose hops,
                    # with the qT copy on ACT to dodge the DVE queue.
                    pjq = ps.tile([64, 128], f32, tag="pj", name=f"pjq{tt}", bufs=3)
                    for c in range(NC_):
                        nc.tensor.matmul(pjq[:], w_sb[:, 192 * c:192 * c + 64],
                                         xsrc(tt, c),
                                         start=(c == 0), stop=(c == NC_ - 1))
                    nc.vector.tensor_copy(qT[:, 128 * tt:128 * (tt + 1)],
                                          pjq[:])
                    return
                wid = 192 if is_key else 64
                pj = ps.tile([128, wid], f32, tag="pj", name=f"pj{tt}", bufs=3)
                for c in range(NC_):
                    nc.tensor.matmul(pj[:], xsrc(tt, c),
                                     w_sb[:, 192 * c:192 * c + wid],
                                     start=(c == 0), stop=(c == NC_ - 1))
                qk = qkp.tile([128, 128 if is_key else 64], bf16, tag="qk",
                              name=f"qk{tt}")
                if is_key:
                    j = tt // 2
                    nc.vector.tensor_copy(qk[:], pj[:, 0:128])
                    nc.vector.tensor_copy(v_sb[:, 65 * j:65 * j + 64],
                                          pj[:, 128:192])
                else:
                    nc.vector.tensor_copy(qk[:], pj[:])
                tp = ps.tile([64, 256], bf16, tag="pj", name=f"tp{tt}", bufs=3)
                if proj_prev[0] is not None:
                    proj_tp(proj_prev[0])
                proj_prev[0] = (tt, qk, tp)

            def proj_flush():
                if proj_prev[0] is not None:
                    proj_tp(proj_prev[0])
                    proj_prev[0] = None

            # ---- attention: one continuous pair stream across supertiles ----
            # Global software pipeline: the U matmuls of a pair are deferred
            # two pairs (possibly crossing into the next supertile) so the
            # PE never sits on the ACT exp critical path, and the ACT stream
            # has no supertile-boundary bubble.
            u_state = {}   # st -> (u4 tile, n_emitted)
            pendings = []  # (st, jp, p2)

            def emit_u_d(st, jp, p2, d):
                if st not in u_state:
                    u_state[st] = [ps.tile([128, 260], f32, tag="u",
                                           name=f"u{st}", bufs=1), 0]
                ent = u_state[st]
                u4 = ent[0]
                j = 2 * jp + d
                total = 8 * (st + 1) - 2  # diag d1 contributes only g=2,3
                if jp == st and d == 1:
                    gs = [(2, 512), (3, 640)]
                else:
                    gs = [(g, 512 * d + 128 * g) for g in range(4)]
                for g, lo in gs:
                    # start=True zeroes the WHOLE PSUM bank: set it only
                    # on the chronologically first matmul into u4.
                    nc.tensor.matmul(
                        u4[:, 65 * g:65 * (g + 1)],
                        p2[:, lo:lo + 128],
                        v_sb[:, 65 * j:65 * (j + 1)],
                        start=(ent[1] == 0),
                        stop=(ent[1] == total - 1),
                        skip_group_check=True)
                    ent[1] += 1

            def ship(st):
                ent = u_state[st]
                if ent[1] == 8 * (st + 1) - 2:  # supertile complete -> ship
                    uo_t = uop.tile([128, 260], f32, tag="uo", name=f"uo{st}")
                    if st <= 2:
                        nc.scalar.activation(uo_t[:], ent[0][:],
                                             mybir.ActivationFunctionType.Copy)
                    else:
                        nc.vector.tensor_copy(uo_t[:], ent[0][:])
                    eng = nc.sync if st == NST - 1 else nc.gpsimd
                    eng.dma_start(out[:, 260 * st:260 * (st + 1)], uo_t[:])
                    del u_state[st]

            def emit_u(st, jp, p2):
                if jp == st:  # diagonal pair -> causal mask, deferred here so
                    # the DVE is free for critical copies at the boundary
                    nc.vector.tensor_mul(p2[:, 0:768], p2[:, 0:768], msk_sb)
                emit_u_d(st, jp, p2, 0)
                emit_u_d(st, jp, p2, 1)
                ship(st)

            def emit_pair(st, jp):
                qsl = slice(STQ * st, STQ * (st + 1))
                s2 = ps.tile([128, 1024], f32, tag="s", name=f"s{st}_{jp}")
                p2 = pp.tile([128, 1024], bf16, tag="p", name=f"p{st}_{jp}")
                if jp == st:
                    # diagonal pair: with 128-interleaved keys, the second
                    # tile is visible only to queries [256:512) for BOTH
                    # halves -> 768 live columns instead of 1024
                    nc.tensor.matmul(s2[:, 0:512],
                                     kT[:, 128 * 2 * jp:128 * (2 * jp + 1)],
                                     qT[:, qsl], start=True, stop=True)
                    nc.tensor.matmul(s2[:, 512:768],
                                     kT[:, 128 * (2 * jp + 1):128 * (2 * jp + 2)],
                                     qT[:, STQ * st + 256:STQ * (st + 1)],
                                     start=True, stop=True)
                    nc.scalar.activation(p2[:, 0:768], s2[:, 0:768],
                                         mybir.ActivationFunctionType.Exp,
                                         scale=0.125)
                else:
                    for d in range(2):
                        j = 2 * jp + d
                        nc.tensor.matmul(s2[:, 512 * d:512 * (d + 1)],
                                         kT[:, 128 * j:128 * (j + 1)],
                                         qT[:, qsl], start=True, stop=True)
                    nc.scalar.activation(p2[:], s2[:],
                                         mybir.ActivationFunctionType.Exp,
                                         scale=0.125)
                pendings.append((st, jp, p2))
                if len(pendings) > 5:
                    emit_u(*pendings.pop(0))

            def filler_q(qtr, half):
                return deque(
                    (lambda t: (lambda: proj_unit(t)))(8 * qtr + 4 * half + i)
                    for i in range(4))

            # Fillers (projection units of supertile st+1) are emitted inside
            # supertile st's pair loop, starting at the pair index where their
            # DMA chunk has arrived (the stream is DMA-paced early on).
            for tl in range(4):
                proj_unit(tl)
            schedule = [filler_q(0, 1), filler_q(1, 0), filler_q(1, 1),
                        filler_q(2, 0), filler_q(2, 1), filler_q(3, 0),
                        filler_q(3, 1), deque()]
            for st in range(8):
                fillers = schedule[st]
                proj_flush()  # this supertile's Q^T/K^T must be complete
                for pi, jp in enumerate([st] + list(range(st))):  # diag first
                    emit_pair(st, jp)
                    if fillers:
                        fillers.popleft()()
                while fillers:
                    fillers.popleft()()
            for pd in pendings:
                emit_u(*pd)
            pendings.clear()

    nc.compile()
    return nc


def _get_nc():
    if "nc" not in _CACHE:
        _CACHE["nc"] = _build()
    return _CACHE["nc"]


def kernel(x, Wq, Wk, Wv, _trace=False):
    x = np.asarray(x)
    nc = _get_nc()

    # Token permutation per half: the program treats EVEN 128-token tiles
    # as key tiles.  For h=1 cores we swap each adjacent tile pair so THEIR
    # key tiles land on even positions.
    tok = np.arange(T)
    perm1 = 128 * ((tok // 128) ^ 1) + tok % 128  # swap adjacent 128-tiles

    xT = np.ascontiguousarray(x.transpose(0, 2, 1)).astype(BF)   # [B, C, T]
    xT1 = np.ascontiguousarray(xT[:, :, perm1])

    w_all = np.concatenate([Wq, Wk, Wv], axis=1).astype(np.float32)  # [C, 192]
    w_packed = np.ascontiguousarray(
        w_all.reshape(NC_, 128, 192).transpose(1, 0, 2).reshape(128, NC_ * 192)
    ).astype(BF)
    idn = np.eye(128, dtype=BF)

    # Masks for the diagonal pair: program key tile A holds global tile
    # 4st+h, tile B holds 4st+2+h; program query quarter g holds global
    # tile 4st+(g^h).  Causal test on global ids:
    #   A: 128h + k <= 128(g^h) + i   (cols 0:512, all four quarters)
    #   B: 128(2+h) + k <= 128(g^h) + i   (cols 512:768, quarters g=2,3)
    i = np.arange(128)[None, :]
    k = np.arange(128)[:, None]
    masks = {}
    for h in range(2):
        colsA = [(128 * h + k <= 128 * (g ^ h) + i) for g in range(4)]
        colsB = [(128 * (2 + h) + k <= 128 * (g ^ h) + i) for g in (2, 3)]
        masks[h] = np.concatenate(colsA + colsB, axis=1).astype(BF)  # [128,768]

    def pack_tiles(xTb):
        # [C, T] -> [128, tt*1024 + c*128 + t]
        return np.ascontiguousarray(
            xTb.reshape(NC_, 128, NTT, 128).transpose(1, 2, 0, 3)
            .reshape(128, NTT * 1024))

    in_maps = []
    for c in range(8):
        b, h = c % 4, c // 4
        xTb = xT[b] if h == 0 else xT1[b]
        in_maps.append({
            "xq": pack_tiles(xTb),
            "w": w_packed,
            "aux2": np.concatenate([masks[h], idn], axis=1),
        })

    res = bass_utils.run_bass_kernel_spmd(nc, in_maps, core_ids=list(range(8)),
                                          trace=_trace)
    _CACHE["last_results"] = res

    # Decode: U[c] [128, 8*260] -> [q_perm, 65]; un-permute h=1 tokens.
    O = np.empty((B, T, D), dtype=np.float32)
    for b in range(B):
        Uh = []
        for h in range(2):
            U = res.results[b + 4 * h]["out"]            # [128, 2080]
            U = U.reshape(128, NST, 4, 65).transpose(1, 2, 0, 3)
            U = U.reshape(T, 65)                          # permuted q order
            Uh.append(U[perm1] if h == 1 else U)          # global q order
        Ut = Uh[0] + Uh[1]
        O[b] = Ut[:, 0:64] / Ut[:, 64:65]
    return O


# revision 12
# speedup vs baseline: 1.1821x; 1.0020x over previous
"""Causal single-head attention (B=4, T=4096, C=1024, D=64) on 8 NeuronCores.

Sharding: core c = (batch b = c % 4, half h = c // 4).
Each core handles ALL queries of its batch against its half of the key
blocks (256-token blocks with block index ≡ h mod 2).  Pure SPMD: the
program is identical on every core; cores differ only in input data
(x[b]^T, block-pair-permuted for h=1, and the causal mask).  Each core
emits unnormalized U[q, 0:64] = sum_k exp(s) v and U[q, 64] = sum_k
exp(s); the host adds the two halves per batch and normalizes.

On-chip dataflow (bf16 compute, f32 PSUM accumulation):
  per 128-token tile tt:  PJ [128t, 192] = sum_c x_tile[c,tt]^T @ w[c]
    (fused Q|K|V projection, x tile stationary; non-key tiles: Q only)
  Q^T, K^T via PE transposes (deferred one unit for pipelining);
  V kept in [token, d] layout + ones column (softmax denominator)
  per query supertile st (512 q), local key tile pairs jp (diag first):
    S^T [128k, 2x512q] = K^T[j] @ Q^T[st]     (contraction over D=64)
    P^T = exp(S^T / 8)                        (one ACT instr per pair)
    diagonal pair: P^T *= mask                (DVE)
    U [128q, 65] += P^T-chunk^T @ [V_j | 1]   (P chunk stationary,
        deferred TWO pairs so the PE stays off the exp critical path)
  Projection units of the NEXT supertile interleave with attention
  pairs as fillers.  U -> SBUF -> DMA out per supertile (SWDGE/Pool
  path, keeping HWDGE free for input streaming).
"""
import sys
import numpy as np
import ml_dtypes
from collections import deque

if "/opt/trn_rl_repo" not in sys.path:
    sys.path.insert(0, "/opt/trn_rl_repo")

import concourse.bacc as bacc
import concourse.mybir as mybir
from concourse import tile
from concourse import bass_utils

bf16 = mybir.dt.bfloat16
f32 = mybir.dt.float32
BF = ml_dtypes.bfloat16

B, T, C, D = 4, 4096, 1024, 64
NC_ = C // 128      # 8 c-tiles
NTT = T // 128      # 32 token tiles
NST = 8             # query supertiles (512 q each)
STQ = 512

_CACHE = {}


def _build():
    nc = bacc.Bacc(None, target_bir_lowering=False, debug=False, num_devices=8)

    # xq tile-packed: xq[:, 1024*tt + 128*c : +128] = x^T[c-tile, token tile tt]
    xq = nc.dram_tensor("xq", [128, NTT * 1024], bf16, kind="ExternalInput")
    w = nc.dram_tensor("w", [128, NC_ * 192], bf16, kind="ExternalInput")
    # aux2 = diag mask [0:768] | identity [768:896]
    aux2 = nc.dram_tensor("aux2", [128, 896], bf16, kind="ExternalInput")
    out = nc.dram_tensor("out", [128, NST * 260], f32, kind="ExternalOutput")

    with tile.TileContext(nc) as tc:
        with tc.tile_pool(name="sb", bufs=1) as sb, \
             tc.tile_pool(name="qk", bufs=3) as qkp, \
             tc.tile_pool(name="pp", bufs=6) as pp, \
             tc.tile_pool(name="uo", bufs=2) as uop, \
             tc.tile_pool(name="ps", bufs=2, space="PSUM") as ps:

            # ---- resident tiles ----
            xq_sb = sb.tile([128, NTT * 1024], bf16, tag="xq")
            w_sb = sb.tile([128, NC_ * 192], bf16, tag="w")
            aux_sb = sb.tile([128, 896], bf16, tag="aux")
            msk_sb = aux_sb[:, 0:768]
            idn_sb = aux_sb[:, 768:896]
            qT = sb.tile([64, T], bf16, tag="qT")       # Q^T strip
            kT = sb.tile([64, T // 2], bf16, tag="kT")  # K^T local tiles
            v_sb = sb.tile([128, 16 * 65], bf16, tag="v")  # [V_j | 1] tiles

            # DMA order = consumption order: w, then one DMA per token tile
            # (each delivers ALL c-tiles of that tile, so projection unit tt
            # unblocks as soon as ITS chunk lands).
            nc.sync.dma_start(w_sb[:], w[:])
            for tt in range(NTT):
                nc.sync.dma_start(xq_sb[:, 1024 * tt:1024 * (tt + 1)],
                                  xq[:, 1024 * tt:1024 * (tt + 1)])
                if tt == 2:  # identity gates the first transposes (~chunk3)
                    nc.sync.dma_start(aux_sb[:], aux2[:])
            # ones columns of the V tiles (denominator trick)
            for j in range(16):
                nc.gpsimd.memset(v_sb[:, 65 * j + 64:65 * j + 65], 1.0)

            # PE p-state warmup: the PE ramps to full clock only after ~3us
            # of continuous busy; run junk matmuls on a zeroed tile right at
            # t=0 so the ramp completes before the first real projection.
            warm = sb.tile([128, 640], bf16, tag="warm")
            nc.vector.memset(warm[:], 0.0)
            # trigger the ACT Exp table load (1.28us) at t=0, not at the
            # first real exp
            nc.scalar.activation(warm[:, 0:1], warm[:, 0:1],
                                 mybir.ActivationFunctionType.Exp, scale=1.0)
            for i in range(5):
                wps = ps.tile([128, 512], f32, tag="s", name=f"warm{i}")
                nc.tensor.matmul(wps[:], warm[:, 0:128], warm[:, 128:640],
                                 start=True, stop=True)

            # ---- projection units, software-pipelined ----
            # mm-phase: fused projection matmuls into PSUM + DVE copies out.
            # tp-phase (transposes + Q^T/K^T copies) is deferred one unit so
            # the PE never waits on the just-issued DVE copy.
            proj_prev = [None]

            def xsrc(tt, c):
                return xq_sb[:, 1024 * tt + 128 * c:1024 * tt + 128 * (c + 1)]

            def proj_tp(state):
                tt, qk, tp = state
                is_key = (tt & 1) == 0
                if is_key:
                    j = tt // 2
                    nc.tensor.transpose(tp[:, 0:128], qk[:, 0:64], idn_sb)
                    nc.tensor.transpose(tp[:, 128:256], qk[:, 64:128], idn_sb)
                    nc.vector.tensor_copy(qT[:, 128 * tt:128 * (tt + 1)],
                                          tp[:, 0:128])
                    nc.vector.tensor_copy(kT[:, 128 * j:128 * (j + 1)],
                                          tp[:, 128:256])
                else:
                    nc.tensor.transpose(tp[:, 0:128], qk[:], idn_sb)
                    nc.vector.tensor_copy(qT[:, 128 * tt:128 * (tt + 1)],
                                          tp[:, 0:128])

            def proj_unit(tt):
                """Key-ness fixed to tt%2==0 (128-interleaved); h=1 cores get
                adjacent-tile-permuted inputs so their key tiles land on
                even positions."""
                is_key = (tt & 1) == 0
                if not is_key and tt % 2 == 1 and tt <= 19:
                    # The tile that gates attention(st): compute Q^T directly
                    # (w stationary), skipping the qk-copy + transpose hops,
                    # with the qT copy on ACT to dodge the DVE queue.
                    pjq = ps.tile([64, 128], f32, tag="pj", name=f"pjq{tt}", bufs=3)
                    for c in range(NC_):
                        nc.tensor.matmul(pjq[:], w_sb[:, 192 * c:192 * c + 64],
                                         xsrc(tt, c),
                                         start=(c == 0), stop=(c == NC_ - 1))
                    nc.vector.tensor_copy(qT[:, 128 * tt:128 * (tt + 1)],
                                          pjq[:])
                    return
                wid = 192 if is_key else 64
                pj = ps.tile([128, wid], f32, tag="pj", name=f"pj{tt}", bufs=3)
                for c in range(NC_):
                    nc.tensor.matmul(pj[:], xsrc(tt, c),
                                     w_sb[:, 192 * c:192 * c + wid],
                                     start=(c == 0), stop=(c == NC_ - 1))
                qk = qkp.tile([128, 128 if is_key else 64], bf16, tag="qk",
                              name=f"qk{tt}")
                if is_key:
                    j = tt // 2
                    nc.vector.tensor_copy(qk[:], pj[:, 0:128])
                    nc.vector.tensor_copy(v_sb[:, 65 * j:65 * j + 64],
                                          pj[:, 128:192])
                else:
                    nc.vector.tensor_copy(qk[:], pj[:])
                tp = ps.tile([64, 256], bf16, tag="pj", name=f"tp{tt}", bufs=3)
                if proj_prev[0] is not None:
                    proj_tp(proj_prev[0])
                proj_prev[0] = (tt, qk, tp)

            def proj_flush():
                if proj_prev[0] is not None:
                    proj_tp(proj_prev[0])
                    proj_prev[0] = None

            # ---- attention: one continuous pair stream across supertiles ----
            # Global software pipeline: the U matmuls of a pair are deferred
            # two pairs (possibly crossing into the next supertile) so the
            # PE never sits on the ACT exp critical path, and the ACT stream
            # has no supertile-boundary bubble.
            u_state = {}   # st -> (u4 tile, n_emitted)
            pendings = []  # (st, jp, p2)

            def emit_u_d(st, jp, p2, d):
                if st not in u_state:
                    u_state[st] = [ps.tile([128, 260], f32, tag="u",
                                           name=f"u{st}", bufs=1), 0]
                ent = u_state[st]
                u4 = ent[0]
                j = 2 * jp + d
                total = 8 * (st + 1) - 2  # diag d1 contributes only g=2,3
                if jp == st and d == 1:
                    gs = [(2, 512), (3, 640)]
                else:
                    gs = [(g, 512 * d + 128 * g) for g in range(4)]
                for g, lo in gs:
                    # start=True zeroes the WHOLE PSUM bank: set it only
                    # on the chronologically first matmul into u4.
                    nc.tensor.matmul(
                        u4[:, 65 * g:65 * (g + 1)],
                        p2[:, lo:lo + 128],
                        v_sb[:, 65 * j:65 * (j + 1)],
                        start=(ent[1] == 0),
                        stop=(ent[1] == total - 1),
                        skip_group_check=True)
                    ent[1] += 1

            def ship(st):
                ent = u_state[st]
                if ent[1] == 8 * (st + 1) - 2:  # supertile complete -> ship
                    uo_t = uop.tile([128, 260], f32, tag="uo", name=f"uo{st}")
                    if st <= 3:
                        nc.scalar.activation(uo_t[:], ent[0][:],
                                             mybir.ActivationFunctionType.Copy)
                    else:
                        nc.vector.tensor_copy(uo_t[:], ent[0][:])
                    eng = nc.sync if st == NST - 1 else nc.gpsimd
                    eng.dma_start(out[:, 260 * st:260 * (st + 1)], uo_t[:])
                    del u_state[st]

            def emit_u(st, jp, p2):
                if jp == st:  # diagonal pair -> causal mask, deferred here so
                    # the DVE is free for critical copies at the boundary
                    nc.vector.tensor_mul(p2[:, 0:768], p2[:, 0:768], msk_sb)
                emit_u_d(st, jp, p2, 0)
                emit_u_d(st, jp, p2, 1)
                ship(st)

            def emit_pair(st, jp):
                qsl = slice(STQ * st, STQ * (st + 1))
                s2 = ps.tile([128, 1024], f32, tag="s", name=f"s{st}_{jp}")
                p2 = pp.tile([128, 1024], bf16, tag="p", name=f"p{st}_{jp}")
                if jp == st:
                    # diagonal pair: with 128-interleaved keys, the second
                    # tile is visible only to queries [256:512) for BOTH
                    # halves -> 768 live columns instead of 1024
                    nc.tensor.matmul(s2[:, 0:512],
                                     kT[:, 128 * 2 * jp:128 * (2 * jp + 1)],
                                     qT[:, qsl], start=True, stop=True)
                    nc.tensor.matmul(s2[:, 512:768],
                                     kT[:, 128 * (2 * jp + 1):128 * (2 * jp + 2)],
                                     qT[:, STQ * st + 256:STQ * (st + 1)],
                                     start=True, stop=True)
                    nc.scalar.activation(p2[:, 0:768], s2[:, 0:768],
                                         mybir.ActivationFunctionType.Exp,
                                         scale=0.125)
                else:
                    for d in range(2):
                        j = 2 * jp + d
                        nc.tensor.matmul(s2[:, 512 * d:512 * (d + 1)],
                                         kT[:, 128 * j:128 * (j + 1)],
                                         qT[:, qsl], start=True, stop=True)
                    nc.scalar.activation(p2[:], s2[:],
                                         mybir.ActivationFunctionType.Exp,
                                         scale=0.125)
                pendings.append((st, jp, p2))
                if len(pendings) > 5:
                    emit_u(*pendings.pop(0))

            def filler_q(qtr, half):
                return deque(
                    (lambda t: (lambda: proj_unit(t)))(8 * qtr + 4 * half + i)
                    for i in range(4))

            # Fillers (projection units of supertile st+1) are emitted inside
            # supertile st's pair loop, starting at the pair index where their
            # DMA chunk has arrived (the stream is DMA-paced early on).
            for tl in range(4):
                proj_unit(tl)
            schedule = [filler_q(0, 1), filler_q(1, 0), filler_q(1, 1),
                        filler_q(2, 0), filler_q(2, 1), filler_q(3, 0),
                        filler_q(3, 1), deque()]
            for st in range(8):
                fillers = schedule[st]
                proj_flush()  # this supertile's Q^T/K^T must be complete
                for pi, jp in enumerate([st] + list(range(st))):  # diag first
                    emit_pair(st, jp)
                    if fillers:
                        fillers.popleft()()
                while fillers:
                    fillers.popleft()()
            for pd in pendings:
                emit_u(*pd)
            pendings.clear()

    nc.compile()
    return nc


def _get_nc():
    if "nc" not in _CACHE:
        _CACHE["nc"] = _build()
    return _CACHE["nc"]


def kernel(x, Wq, Wk, Wv, _trace=False):
    x = np.asarray(x)
    nc = _get_nc()

    # Token permutation per half: the program treats EVEN 128-token tiles
    # as key tiles.  For h=1 cores we swap each adjacent tile pair so THEIR
    # key tiles land on even positions.
    tok = np.arange(T)
    perm1 = 128 * ((tok // 128) ^ 1) + tok % 128  # swap adjacent 128-tiles

    xT = np.ascontiguousarray(x.transpose(0, 2, 1)).astype(BF)   # [B, C, T]
    xT1 = np.ascontiguousarray(xT[:, :, perm1])

    w_all = np.concatenate([Wq, Wk, Wv], axis=1).astype(np.float32)  # [C, 192]
    w_packed = np.ascontiguousarray(
        w_all.reshape(NC_, 128, 192).transpose(1, 0, 2).reshape(128, NC_ * 192)
    ).astype(BF)
    idn = np.eye(128, dtype=BF)

    # Masks for the diagonal pair: program key tile A holds global tile
    # 4st+h, tile B holds 4st+2+h; program query quarter g holds global
    # tile 4st+(g^h).  Causal test on global ids:
    #   A: 128h + k <= 128(g^h) + i   (cols 0:512, all four quarters)
    #   B: 128(2+h) + k <= 128(g^h) + i   (cols 512:768, quarters g=2,3)
    i = np.arange(128)[None, :]
    k = np.arange(128)[:, None]
    masks = {}
    for h in range(2):
        colsA = [(128 * h + k <= 128 * (g ^ h) + i) for g in range(4)]
        colsB = [(128 * (2 + h) + k <= 128 * (g ^ h) + i) for g in (2, 3)]
        masks[h] = np.concatenate(colsA + colsB, axis=1).astype(BF)  # [128,768]

    def pack_tiles(xTb):
        # [C, T] -> [128, tt*1024 + c*128 + t]
        return np.ascontiguousarray(
            xTb.reshape(NC_, 128, NTT, 128).transpose(1, 2, 0, 3)
            .reshape(128, NTT * 1024))

    in_maps = []
    for c in range(8):
        b, h = c % 4, c // 4
        xTb = xT[b] if h == 0 else xT1[b]
        in_maps.append({
            "xq": pack_tiles(xTb),
            "w": w_packed,
            "aux2": np.concatenate([masks[h], idn], axis=1),
        })

    res = bass_utils.run_bass_kernel_spmd(nc, in_maps, core_ids=list(range(8)),
                                          trace=_trace)
    _CACHE["last_results"] = res

    # Decode: U[c] [128, 8*260] -> [q_perm, 65]; un-permute h=1 tokens.
    O = np.empty((B, T, D), dtype=np.float32)
    for b in range(B):
        Uh = []
        for h in range(2):
            U = res.results[b + 4 * h]["out"]            # [128, 2080]
            U = U.reshape(128, NST, 4, 65).transpose(1, 2, 0, 3)
            U = U.reshape(T, 65)                          # permuted q order
            Uh.append(U[perm1] if h == 1 else U)          # global q order
        Ut = Uh[0] + Uh[1]
        O[b] = Ut[:, 0:64] / Ut[:, 64:65]
    return O


# revision 13
# speedup vs baseline: 1.1830x; 1.0008x over previous
"""Causal single-head attention (B=4, T=4096, C=1024, D=64) on 8 NeuronCores.

Sharding: core c = (batch b = c % 4, half h = c // 4).
Each core handles ALL queries of its batch against its half of the key
blocks (256-token blocks with block index ≡ h mod 2).  Pure SPMD: the
program is identical on every core; cores differ only in input data
(x[b]^T, block-pair-permuted for h=1, and the causal mask).  Each core
emits unnormalized U[q, 0:64] = sum_k exp(s) v and U[q, 64] = sum_k
exp(s); the host adds the two halves per batch and normalizes.

On-chip dataflow (bf16 compute, f32 PSUM accumulation):
  per 128-token tile tt:  PJ [128t, 192] = sum_c x_tile[c,tt]^T @ w[c]
    (fused Q|K|V projection, x tile stationary; non-key tiles: Q only)
  Q^T, K^T via PE transposes (deferred one unit for pipelining);
  V kept in [token, d] layout + ones column (softmax denominator)
  per query supertile st (512 q), local key tile pairs jp (diag first):
    S^T [128k, 2x512q] = K^T[j] @ Q^T[st]     (contraction over D=64)
    P^T = exp(S^T / 8)                        (one ACT instr per pair)
    diagonal pair: P^T *= mask                (DVE)
    U [128q, 65] += P^T-chunk^T @ [V_j | 1]   (P chunk stationary,
        deferred TWO pairs so the PE stays off the exp critical path)
  Projection units of the NEXT supertile interleave with attention
  pairs as fillers.  U -> SBUF -> DMA out per supertile (SWDGE/Pool
  path, keeping HWDGE free for input streaming).
"""
import sys
import numpy as np
import ml_dtypes
from collections import deque

if "/opt/trn_rl_repo" not in sys.path:
    sys.path.insert(0, "/opt/trn_rl_repo")

import concourse.bacc as bacc
import concourse.mybir as mybir
from concourse import tile
from concourse import bass_utils

bf16 = mybir.dt.bfloat16
f32 = mybir.dt.float32
BF = ml_dtypes.bfloat16

B, T, C, D = 4, 4096, 1024, 64
NC_ = C // 128      # 8 c-tiles
NTT = T // 128      # 32 token tiles
NST = 8             # query supertiles (512 q each)
STQ = 512

_CACHE = {}


def _build():
    nc = bacc.Bacc(None, target_bir_lowering=False, debug=False, num_devices=8)

    # xq tile-packed: xq[:, 1024*tt + 128*c : +128] = x^T[c-tile, token tile tt]
    xq = nc.dram_tensor("xq", [128, NTT * 1024], bf16, kind="ExternalInput")
    w = nc.dram_tensor("w", [128, NC_ * 192], bf16, kind="ExternalInput")
    # aux2 = diag mask [0:768] | identity [768:896]
    aux2 = nc.dram_tensor("aux2", [128, 896], bf16, kind="ExternalInput")
    out = nc.dram_tensor("out", [128, NST * 260], f32, kind="ExternalOutput")

    with tile.TileContext(nc) as tc:
        with tc.tile_pool(name="sb", bufs=1) as sb, \
             tc.tile_pool(name="qk", bufs=3) as qkp, \
             tc.tile_pool(name="pp", bufs=7) as pp, \
             tc.tile_pool(name="uo", bufs=2) as uop, \
             tc.tile_pool(name="ps", bufs=2, space="PSUM") as ps:

            # ---- resident tiles ----
            xq_sb = sb.tile([128, NTT * 1024], bf16, tag="xq")
            w_sb = sb.tile([128, NC_ * 192], bf16, tag="w")
            aux_sb = sb.tile([128, 896], bf16, tag="aux")
            msk_sb = aux_sb[:, 0:768]
            idn_sb = aux_sb[:, 768:896]
            qT = sb.tile([64, T], bf16, tag="qT")       # Q^T strip
            kT = sb.tile([64, T // 2], bf16, tag="kT")  # K^T local tiles
            v_sb = sb.tile([128, 16 * 65], bf16, tag="v")  # [V_j | 1] tiles

            # DMA order = consumption order: w, then one DMA per token tile
            # (each delivers ALL c-tiles of that tile, so projection unit tt
            # unblocks as soon as ITS chunk lands).
            nc.sync.dma_start(w_sb[:], w[:])
            for tt in range(NTT):
                nc.sync.dma_start(xq_sb[:, 1024 * tt:1024 * (tt + 1)],
                                  xq[:, 1024 * tt:1024 * (tt + 1)])
                if tt == 2:  # identity gates the first transposes (~chunk3)
                    nc.sync.dma_start(aux_sb[:], aux2[:])
            # ones columns of the V tiles (denominator trick)
            for j in range(16):
                nc.gpsimd.memset(v_sb[:, 65 * j + 64:65 * j + 65], 1.0)

            # PE p-state warmup: the PE ramps to full clock only after ~3us
            # of continuous busy; run junk matmuls on a zeroed tile right at
            # t=0 so the ramp completes before the first real projection.
            warm = sb.tile([128, 640], bf16, tag="warm")
            nc.vector.memset(warm[:], 0.0)
            # trigger the ACT Exp table load (1.28us) at t=0, not at the
            # first real exp
            nc.scalar.activation(warm[:, 0:1], warm[:, 0:1],
                                 mybir.ActivationFunctionType.Exp, scale=1.0)
            for i in range(5):
                wps = ps.tile([128, 512], f32, tag="s", name=f"warm{i}")
                nc.tensor.matmul(wps[:], warm[:, 0:128], warm[:, 128:640],
                                 start=True, stop=True)

            # ---- projection units, software-pipelined ----
            # mm-phase: fused projection matmuls into PSUM + DVE copies out.
            # tp-phase (transposes + Q^T/K^T copies) is deferred one unit so
            # the PE never waits on the just-issued DVE copy.
            proj_prev = [None]

            def xsrc(tt, c):
                return xq_sb[:, 1024 * tt + 128 * c:1024 * tt + 128 * (c + 1)]

            def proj_tp(state):
                tt, qk, tp = state
                is_key = (tt & 1) == 0
                if is_key:
                    j = tt // 2
                    nc.tensor.transpose(tp[:, 0:128], qk[:, 0:64], idn_sb)
                    nc.tensor.transpose(tp[:, 128:256], qk[:, 64:128], idn_sb)
                    nc.vector.tensor_copy(qT[:, 128 * tt:128 * (tt + 1)],
                                          tp[:, 0:128])
                    nc.vector.tensor_copy(kT[:, 128 * j:128 * (j + 1)],
                                          tp[:, 128:256])
                else:
                    nc.tensor.transpose(tp[:, 0:128], qk[:], idn_sb)
                    nc.vector.tensor_copy(qT[:, 128 * tt:128 * (tt + 1)],
                                          tp[:, 0:128])

            def proj_unit(tt):
                """Key-ness fixed to tt%2==0 (128-interleaved); h=1 cores get
                adjacent-tile-permuted inputs so their key tiles land on
                even positions."""
                is_key = (tt & 1) == 0
                if not is_key and tt % 2 == 1 and tt <= 19:
                    # The tile that gates attention(st): compute Q^T directly
                    # (w stationary), skipping the qk-copy + transpose hops,
                    # with the qT copy on ACT to dodge the DVE queue.
                    pjq = ps.tile([64, 128], f32, tag="pj", name=f"pjq{tt}", bufs=3)
                    for c in range(NC_):
                        nc.tensor.matmul(pjq[:], w_sb[:, 192 * c:192 * c + 64],
                                         xsrc(tt, c),
                                         start=(c == 0), stop=(c == NC_ - 1))
                    nc.vector.tensor_copy(qT[:, 128 * tt:128 * (tt + 1)],
                                          pjq[:])
                    return
                wid = 192 if is_key else 64
                pj = ps.tile([128, wid], f32, tag="pj", name=f"pj{tt}", bufs=3)
                for c in range(NC_):
                    nc.tensor.matmul(pj[:], xsrc(tt, c),
                                     w_sb[:, 192 * c:192 * c + wid],
                                     start=(c == 0), stop=(c == NC_ - 1))
                qk = qkp.tile([128, 128 if is_key else 64], bf16, tag="qk",
                              name=f"qk{tt}")
                if is_key:
                    j = tt // 2
                    nc.vector.tensor_copy(qk[:], pj[:, 0:128])
                    nc.vector.tensor_copy(v_sb[:, 65 * j:65 * j + 64],
                                          pj[:, 128:192])
                else:
                    nc.vector.tensor_copy(qk[:], pj[:])
                tp = ps.tile([64, 256], bf16, tag="pj", name=f"tp{tt}", bufs=3)
                if proj_prev[0] is not None:
                    proj_tp(proj_prev[0])
                proj_prev[0] = (tt, qk, tp)

            def proj_flush():
                if proj_prev[0] is not None:
                    proj_tp(proj_prev[0])
                    proj_prev[0] = None

            # ---- attention: one continuous pair stream across supertiles ----
            # Global software pipeline: the U matmuls of a pair are deferred
            # two pairs (possibly crossing into the next supertile) so the
            # PE never sits on the ACT exp critical path, and the ACT stream
            # has no supertile-boundary bubble.
            u_state = {}   # st -> (u4 tile, n_emitted)
            pendings = []  # (st, jp, p2)

            def emit_u_d(st, jp, p2, d):
                if st not in u_state:
                    u_state[st] = [ps.tile([128, 260], f32, tag="u",
                                           name=f"u{st}", bufs=1), 0]
                ent = u_state[st]
                u4 = ent[0]
                j = 2 * jp + d
                total = 8 * (st + 1) - 2  # diag d1 contributes only g=2,3
                if jp == st and d == 1:
                    gs = [(2, 512), (3, 640)]
                else:
                    gs = [(g, 512 * d + 128 * g) for g in range(4)]
                for g, lo in gs:
                    # start=True zeroes the WHOLE PSUM bank: set it only
                    # on the chronologically first matmul into u4.
                    nc.tensor.matmul(
                        u4[:, 65 * g:65 * (g + 1)],
                        p2[:, lo:lo + 128],
                        v_sb[:, 65 * j:65 * (j + 1)],
                        start=(ent[1] == 0),
                        stop=(ent[1] == total - 1),
                        skip_group_check=True)
                    ent[1] += 1

            def ship(st):
                ent = u_state[st]
                if ent[1] == 8 * (st + 1) - 2:  # supertile complete -> ship
                    uo_t = uop.tile([128, 260], f32, tag="uo", name=f"uo{st}")
                    if st <= 3:
                        nc.scalar.activation(uo_t[:], ent[0][:],
                                             mybir.ActivationFunctionType.Copy)
                    else:
                        nc.vector.tensor_copy(uo_t[:], ent[0][:])
                    eng = nc.sync if st == NST - 1 else nc.gpsimd
                    eng.dma_start(out[:, 260 * st:260 * (st + 1)], uo_t[:])
                    del u_state[st]

            def emit_u(st, jp, p2):
                if jp == st:  # diagonal pair -> causal mask, deferred here so
                    # the DVE is free for critical copies at the boundary
                    nc.vector.tensor_mul(p2[:, 0:768], p2[:, 0:768], msk_sb)
                emit_u_d(st, jp, p2, 0)
                emit_u_d(st, jp, p2, 1)
                ship(st)

            def emit_pair(st, jp):
                qsl = slice(STQ * st, STQ * (st + 1))
                s2 = ps.tile([128, 1024], f32, tag="s", name=f"s{st}_{jp}")
                p2 = pp.tile([128, 1024], bf16, tag="p", name=f"p{st}_{jp}")
                if jp == st:
                    # diagonal pair: with 128-interleaved keys, the second
                    # tile is visible only to queries [256:512) for BOTH
                    # halves -> 768 live columns instead of 1024
                    nc.tensor.matmul(s2[:, 0:512],
                                     kT[:, 128 * 2 * jp:128 * (2 * jp + 1)],
                                     qT[:, qsl], start=True, stop=True)
                    nc.tensor.matmul(s2[:, 512:768],
                                     kT[:, 128 * (2 * jp + 1):128 * (2 * jp + 2)],
                                     qT[:, STQ * st + 256:STQ * (st + 1)],
                                     start=True, stop=True)
                    nc.scalar.activation(p2[:, 0:768], s2[:, 0:768],
                                         mybir.ActivationFunctionType.Exp,
                                         scale=0.125)
                else:
                    for d in range(2):
                        j = 2 * jp + d
                        nc.tensor.matmul(s2[:, 512 * d:512 * (d + 1)],
                                         kT[:, 128 * j:128 * (j + 1)],
                                         qT[:, qsl], start=True, stop=True)
                    nc.scalar.activation(p2[:], s2[:],
                                         mybir.ActivationFunctionType.Exp,
                                         scale=0.125)
                pendings.append((st, jp, p2))
                if len(pendings) > 5:
                    emit_u(*pendings.pop(0))

            def filler_q(qtr, half):
                return deque(
                    (lambda t: (lambda: proj_unit(t)))(8 * qtr + 4 * half + i)
                    for i in range(4))

            # Fillers (projection units of supertile st+1) are emitted inside
            # supertile st's pair loop, starting at the pair index where their
            # DMA chunk has arrived (the stream is DMA-paced early on).
            for tl in range(4):
                proj_unit(tl)
            schedule = [filler_q(0, 1), filler_q(1, 0), filler_q(1, 1),
                        filler_q(2, 0), filler_q(2, 1), filler_q(3, 0),
                        filler_q(3, 1), deque()]
            for st in range(8):
                fillers = schedule[st]
                proj_flush()  # this supertile's Q^T/K^T must be complete
                for pi, jp in enumerate([st] + list(range(st))):  # diag first
                    emit_pair(st, jp)
                    if fillers:
                        fillers.popleft()()
                while fillers:
                    fillers.popleft()()
            for pd in pendings:
                emit_u(*pd)
            pendings.clear()

    nc.compile()
    return nc


def _get_nc():
    if "nc" not in _CACHE:
        _CACHE["nc"] = _build()
    return _CACHE["nc"]


def kernel(x, Wq, Wk, Wv, _trace=False):
    x = np.asarray(x)
    nc = _get_nc()

    # Token permutation per half: the program treats EVEN 128-token tiles
    # as key tiles.  For h=1 cores we swap each adjacent tile pair so THEIR
    # key tiles land on even positions.
    tok = np.arange(T)
    perm1 = 128 * ((tok // 128) ^ 1) + tok % 128  # swap adjacent 128-tiles

    xT = np.ascontiguousarray(x.transpose(0, 2, 1)).astype(BF)   # [B, C, T]
    xT1 = np.ascontiguousarray(xT[:, :, perm1])

    w_all = np.concatenate([Wq, Wk, Wv], axis=1).astype(np.float32)  # [C, 192]
    w_packed = np.ascontiguousarray(
        w_all.reshape(NC_, 128, 192).transpose(1, 0, 2).reshape(128, NC_ * 192)
    ).astype(BF)
    idn = np.eye(128, dtype=BF)

    # Masks for the diagonal pair: program key tile A holds global tile
    # 4st+h, tile B holds 4st+2+h; program query quarter g holds global
    # tile 4st+(g^h).  Causal test on global ids:
    #   A: 128h + k <= 128(g^h) + i   (cols 0:512, all four quarters)
    #   B: 128(2+h) + k <= 128(g^h) + i   (cols 512:768, quarters g=2,3)
    i = np.arange(128)[None, :]
    k = np.arange(128)[:, None]
    masks = {}
    for h in range(2):
        colsA = [(128 * h + k <= 128 * (g ^ h) + i) for g in range(4)]
        colsB = [(128 * (2 + h) + k <= 128 * (g ^ h) + i) for g in (2, 3)]
        masks[h] = np.concatenate(colsA + colsB, axis=1).astype(BF)  # [128,768]

    def pack_tiles(xTb):
        # [C, T] -> [128, tt*1024 + c*128 + t]
        return np.ascontiguousarray(
            xTb.reshape(NC_, 128, NTT, 128).transpose(1, 2, 0, 3)
            .reshape(128, NTT * 1024))

    in_maps = []
    for c in range(8):
        b, h = c % 4, c // 4
        xTb = xT[b] if h == 0 else xT1[b]
        in_maps.append({
            "xq": pack_tiles(xTb),
            "w": w_packed,
            "aux2": np.concatenate([masks[h], idn], axis=1),
        })

    res = bass_utils.run_bass_kernel_spmd(nc, in_maps, core_ids=list(range(8)),
                                          trace=_trace)
    _CACHE["last_results"] = res

    # Decode: U[c] [128, 8*260] -> [q_perm, 65]; un-permute h=1 tokens.
    O = np.empty((B, T, D), dtype=np.float32)
    for b in range(B):
        Uh = []
        for h in range(2):
            U = res.results[b + 4 * h]["out"]            # [128, 2080]
            U = U.reshape(128, NST, 4, 65).transpose(1, 2, 0, 3)
            U = U.reshape(T, 65)                          # permuted q order
            Uh.append(U[perm1] if h == 1 else U)          # global q order
        Ut = Uh[0] + Uh[1]
        O[b] = Ut[:, 0:64] / Ut[:, 64:65]
    return O


# revision 14
# speedup vs baseline: 1.1869x; 1.0033x over previous
"""Causal single-head attention (B=4, T=4096, C=1024, D=64) on 8 NeuronCores.

Sharding: core c = (batch b = c % 4, half h = c // 4).
Each core handles ALL queries of its batch against its half of the key
blocks (256-token blocks with block index ≡ h mod 2).  Pure SPMD: the
program is identical on every core; cores differ only in input data
(x[b]^T, block-pair-permuted for h=1, and the causal mask).  Each core
emits unnormalized U[q, 0:64] = sum_k exp(s) v and U[q, 64] = sum_k
exp(s); the host adds the two halves per batch and normalizes.

On-chip dataflow (bf16 compute, f32 PSUM accumulation):
  per 128-token tile tt:  PJ [128t, 192] = sum_c x_tile[c,tt]^T @ w[c]
    (fused Q|K|V projection, x tile stationary; non-key tiles: Q only)
  Q^T, K^T via PE transposes (deferred one unit for pipelining);
  V kept in [token, d] layout + ones column (softmax denominator)
  per query supertile st (512 q), local key tile pairs jp (diag first):
    S^T [128k, 2x512q] = K^T[j] @ Q^T[st]     (contraction over D=64)
    P^T = exp(S^T / 8)                        (one ACT instr per pair)
    diagonal pair: P^T *= mask                (DVE)
    U [128q, 65] += P^T-chunk^T @ [V_j | 1]   (P chunk stationary,
        deferred TWO pairs so the PE stays off the exp critical path)
  Projection units of the NEXT supertile interleave with attention
  pairs as fillers.  U -> SBUF -> DMA out per supertile (SWDGE/Pool
  path, keeping HWDGE free for input streaming).
"""
import sys
import numpy as np
import ml_dtypes
from collections import deque

if "/opt/trn_rl_repo" not in sys.path:
    sys.path.insert(0, "/opt/trn_rl_repo")

import concourse.bacc as bacc
import concourse.mybir as mybir
from concourse import tile
from concourse import bass_utils

bf16 = mybir.dt.bfloat16
f32 = mybir.dt.float32
BF = ml_dtypes.bfloat16

B, T, C, D = 4, 4096, 1024, 64
NC_ = C // 128      # 8 c-tiles
NTT = T // 128      # 32 token tiles
NST = 8             # query supertiles (512 q each)
STQ = 512

_CACHE = {}


def _build():
    nc = bacc.Bacc(None, target_bir_lowering=False, debug=False, num_devices=8)

    # xq tile-packed: xq[:, 1024*tt + 128*c : +128] = x^T[c-tile, token tile tt]
    xq = nc.dram_tensor("xq", [128, NTT * 1024], bf16, kind="ExternalInput")
    w = nc.dram_tensor("w", [128, NC_ * 192], bf16, kind="ExternalInput")
    # aux2 = diag mask [0:768] | identity [768:896]
    aux2 = nc.dram_tensor("aux2", [128, 896], bf16, kind="ExternalInput")
    out = nc.dram_tensor("out", [128, NST * 260], bf16, kind="ExternalOutput")

    with tile.TileContext(nc) as tc:
        with tc.tile_pool(name="sb", bufs=1) as sb, \
             tc.tile_pool(name="qk", bufs=3) as qkp, \
             tc.tile_pool(name="pp", bufs=7) as pp, \
             tc.tile_pool(name="uo", bufs=2) as uop, \
             tc.tile_pool(name="ps", bufs=2, space="PSUM") as ps:

            # ---- resident tiles ----
            xq_sb = sb.tile([128, NTT * 1024], bf16, tag="xq")
            w_sb = sb.tile([128, NC_ * 192], bf16, tag="w")
            aux_sb = sb.tile([128, 896], bf16, tag="aux")
            msk_sb = aux_sb[:, 0:768]
            idn_sb = aux_sb[:, 768:896]
            qT = sb.tile([64, T], bf16, tag="qT")       # Q^T strip
            kT = sb.tile([64, T // 2], bf16, tag="kT")  # K^T local tiles
            v_sb = sb.tile([128, 16 * 65], bf16, tag="v")  # [V_j | 1] tiles

            # DMA order = consumption order: w, then one DMA per token tile
            # (each delivers ALL c-tiles of that tile, so projection unit tt
            # unblocks as soon as ITS chunk lands).
            nc.sync.dma_start(w_sb[:], w[:])
            for tt in range(NTT):
                nc.sync.dma_start(xq_sb[:, 1024 * tt:1024 * (tt + 1)],
                                  xq[:, 1024 * tt:1024 * (tt + 1)])
                if tt == 2:  # identity gates the first transposes (~chunk3)
                    nc.sync.dma_start(aux_sb[:], aux2[:])
            # ones columns of the V tiles (denominator trick)
            for j in range(16):
                nc.gpsimd.memset(v_sb[:, 65 * j + 64:65 * j + 65], 1.0)

            # PE p-state warmup: the PE ramps to full clock only after ~3us
            # of continuous busy; run junk matmuls on a zeroed tile right at
            # t=0 so the ramp completes before the first real projection.
            warm = sb.tile([128, 640], bf16, tag="warm")
            nc.vector.memset(warm[:], 0.0)
            # trigger the ACT Exp table load (1.28us) at t=0, not at the
            # first real exp
            nc.scalar.activation(warm[:, 0:1], warm[:, 0:1],
                                 mybir.ActivationFunctionType.Exp, scale=1.0)
            for i in range(5):
                wps = ps.tile([128, 512], f32, tag="s", name=f"warm{i}")
                nc.tensor.matmul(wps[:], warm[:, 0:128], warm[:, 128:640],
                                 start=True, stop=True)

            # ---- projection units, software-pipelined ----
            # mm-phase: fused projection matmuls into PSUM + DVE copies out.
            # tp-phase (transposes + Q^T/K^T copies) is deferred one unit so
            # the PE never waits on the just-issued DVE copy.
            proj_prev = [None]

            def xsrc(tt, c):
                return xq_sb[:, 1024 * tt + 128 * c:1024 * tt + 128 * (c + 1)]

            def proj_tp(state):
                tt, qk, tp = state
                is_key = (tt & 1) == 0
                if is_key:
                    j = tt // 2
                    nc.tensor.transpose(tp[:, 0:128], qk[:, 0:64], idn_sb)
                    nc.tensor.transpose(tp[:, 128:256], qk[:, 64:128], idn_sb)
                    nc.vector.tensor_copy(qT[:, 128 * tt:128 * (tt + 1)],
                                          tp[:, 0:128])
                    nc.vector.tensor_copy(kT[:, 128 * j:128 * (j + 1)],
                                          tp[:, 128:256])
                else:
                    nc.tensor.transpose(tp[:, 0:128], qk[:], idn_sb)
                    nc.vector.tensor_copy(qT[:, 128 * tt:128 * (tt + 1)],
                                          tp[:, 0:128])

            def proj_unit(tt):
                """Key-ness fixed to tt%2==0 (128-interleaved); h=1 cores get
                adjacent-tile-permuted inputs so their key tiles land on
                even positions."""
                is_key = (tt & 1) == 0
                if not is_key and tt % 2 == 1 and tt <= 19:
                    # The tile that gates attention(st): compute Q^T directly
                    # (w stationary), skipping the qk-copy + transpose hops,
                    # with the qT copy on ACT to dodge the DVE queue.
                    pjq = ps.tile([64, 128], f32, tag="pj", name=f"pjq{tt}", bufs=3)
                    for c in range(NC_):
                        nc.tensor.matmul(pjq[:], w_sb[:, 192 * c:192 * c + 64],
                                         xsrc(tt, c),
                                         start=(c == 0), stop=(c == NC_ - 1))
                    nc.vector.tensor_copy(qT[:, 128 * tt:128 * (tt + 1)],
                                          pjq[:])
                    return
                wid = 192 if is_key else 64
                pj = ps.tile([128, wid], f32, tag="pj", name=f"pj{tt}", bufs=3)
                for c in range(NC_):
                    nc.tensor.matmul(pj[:], xsrc(tt, c),
                                     w_sb[:, 192 * c:192 * c + wid],
                                     start=(c == 0), stop=(c == NC_ - 1))
                qk = qkp.tile([128, 128 if is_key else 64], bf16, tag="qk",
                              name=f"qk{tt}")
                if is_key:
                    j = tt // 2
                    nc.vector.tensor_copy(qk[:], pj[:, 0:128])
                    nc.vector.tensor_copy(v_sb[:, 65 * j:65 * j + 64],
                                          pj[:, 128:192])
                else:
                    nc.vector.tensor_copy(qk[:], pj[:])
                tp = ps.tile([64, 256], bf16, tag="pj", name=f"tp{tt}", bufs=3)
                if proj_prev[0] is not None:
                    proj_tp(proj_prev[0])
                proj_prev[0] = (tt, qk, tp)

            def proj_flush():
                if proj_prev[0] is not None:
                    proj_tp(proj_prev[0])
                    proj_prev[0] = None

            # ---- attention: one continuous pair stream across supertiles ----
            # Global software pipeline: the U matmuls of a pair are deferred
            # two pairs (possibly crossing into the next supertile) so the
            # PE never sits on the ACT exp critical path, and the ACT stream
            # has no supertile-boundary bubble.
            u_state = {}   # st -> (u4 tile, n_emitted)
            pendings = []  # (st, jp, p2)

            def emit_u_d(st, jp, p2, d):
                if st not in u_state:
                    u_state[st] = [ps.tile([128, 260], f32, tag="u",
                                           name=f"u{st}", bufs=1), 0]
                ent = u_state[st]
                u4 = ent[0]
                j = 2 * jp + d
                total = 8 * (st + 1) - 2  # diag d1 contributes only g=2,3
                if jp == st and d == 1:
                    gs = [(2, 512), (3, 640)]
                else:
                    gs = [(g, 512 * d + 128 * g) for g in range(4)]
                for g, lo in gs:
                    # start=True zeroes the WHOLE PSUM bank: set it only
                    # on the chronologically first matmul into u4.
                    nc.tensor.matmul(
                        u4[:, 65 * g:65 * (g + 1)],
                        p2[:, lo:lo + 128],
                        v_sb[:, 65 * j:65 * (j + 1)],
                        start=(ent[1] == 0),
                        stop=(ent[1] == total - 1),
                        skip_group_check=True)
                    ent[1] += 1

            def ship(st):
                ent = u_state[st]
                if ent[1] == 8 * (st + 1) - 2:  # supertile complete -> ship
                    uo_t = uop.tile([128, 260], bf16, tag="uo", name=f"uo{st}")
                    if st <= 3:
                        nc.scalar.activation(uo_t[:], ent[0][:],
                                             mybir.ActivationFunctionType.Copy)
                    else:
                        nc.vector.tensor_copy(uo_t[:], ent[0][:])
                    eng = nc.sync if st == NST - 1 else nc.gpsimd
                    eng.dma_start(out[:, 260 * st:260 * (st + 1)], uo_t[:])
                    del u_state[st]

            def emit_u(st, jp, p2):
                if jp == st:  # diagonal pair -> causal mask, deferred here so
                    # the DVE is free for critical copies at the boundary
                    nc.vector.tensor_mul(p2[:, 0:768], p2[:, 0:768], msk_sb)
                emit_u_d(st, jp, p2, 0)
                emit_u_d(st, jp, p2, 1)
                ship(st)

            def emit_pair(st, jp):
                qsl = slice(STQ * st, STQ * (st + 1))
                s2 = ps.tile([128, 1024], f32, tag="s", name=f"s{st}_{jp}")
                p2 = pp.tile([128, 1024], bf16, tag="p", name=f"p{st}_{jp}")
                if jp == st:
                    # diagonal pair: with 128-interleaved keys, the second
                    # tile is visible only to queries [256:512) for BOTH
                    # halves -> 768 live columns instead of 1024
                    nc.tensor.matmul(s2[:, 0:512],
                                     kT[:, 128 * 2 * jp:128 * (2 * jp + 1)],
                                     qT[:, qsl], start=True, stop=True)
                    nc.tensor.matmul(s2[:, 512:768],
                                     kT[:, 128 * (2 * jp + 1):128 * (2 * jp + 2)],
                                     qT[:, STQ * st + 256:STQ * (st + 1)],
                                     start=True, stop=True)
                    nc.scalar.activation(p2[:, 0:768], s2[:, 0:768],
                                         mybir.ActivationFunctionType.Exp,
                                         scale=0.125)
                else:
                    for d in range(2):
                        j = 2 * jp + d
                        nc.tensor.matmul(s2[:, 512 * d:512 * (d + 1)],
                                         kT[:, 128 * j:128 * (j + 1)],
                                         qT[:, qsl], start=True, stop=True)
                    nc.scalar.activation(p2[:], s2[:],
                                         mybir.ActivationFunctionType.Exp,
                                         scale=0.125)
                pendings.append((st, jp, p2))
                if len(pendings) > 5:
                    emit_u(*pendings.pop(0))

            def filler_q(qtr, half):
                return deque(
                    (lambda t: (lambda: proj_unit(t)))(8 * qtr + 4 * half + i)
                    for i in range(4))

            # Fillers (projection units of supertile st+1) are emitted inside
            # supertile st's pair loop, starting at the pair index where their
            # DMA chunk has arrived (the stream is DMA-paced early on).
            for tl in range(4):
                proj_unit(tl)
            schedule = [filler_q(0, 1), filler_q(1, 0), filler_q(1, 1),
                        filler_q(2, 0), filler_q(2, 1), filler_q(3, 0),
                        filler_q(3, 1), deque()]
            for st in range(8):
                fillers = schedule[st]
                proj_flush()  # this supertile's Q^T/K^T must be complete
                for pi, jp in enumerate([st] + list(range(st))):  # diag first
                    emit_pair(st, jp)
                    if fillers:
                        fillers.popleft()()
                while fillers:
                    fillers.popleft()()
            for pd in pendings:
                emit_u(*pd)
            pendings.clear()

    nc.compile()
    return nc


def _get_nc():
    if "nc" not in _CACHE:
        _CACHE["nc"] = _build()
    return _CACHE["nc"]


def kernel(x, Wq, Wk, Wv, _trace=False):
    x = np.asarray(x)
    nc = _get_nc()

    # Token permutation per half: the program treats EVEN 128-token tiles
    # as key tiles.  For h=1 cores we swap each adjacent tile pair so THEIR
    # key tiles land on even positions.
    tok = np.arange(T)
    perm1 = 128 * ((tok // 128) ^ 1) + tok % 128  # swap adjacent 128-tiles

    xT = np.ascontiguousarray(x.transpose(0, 2, 1)).astype(BF)   # [B, C, T]
    xT1 = np.ascontiguousarray(xT[:, :, perm1])

    w_all = np.concatenate([Wq, Wk, Wv], axis=1).astype(np.float32)  # [C, 192]
    w_packed = np.ascontiguousarray(
        w_all.reshape(NC_, 128, 192).transpose(1, 0, 2).reshape(128, NC_ * 192)
    ).astype(BF)
    idn = np.eye(128, dtype=BF)

    # Masks for the diagonal pair: program key tile A holds global tile
    # 4st+h, tile B holds 4st+2+h; program query quarter g holds global
    # tile 4st+(g^h).  Causal test on global ids:
    #   A: 128h + k <= 128(g^h) + i   (cols 0:512, all four quarters)
    #   B: 128(2+h) + k <= 128(g^h) + i   (cols 512:768, quarters g=2,3)
    i = np.arange(128)[None, :]
    k = np.arange(128)[:, None]
    masks = {}
    for h in range(2):
        colsA = [(128 * h + k <= 128 * (g ^ h) + i) for g in range(4)]
        colsB = [(128 * (2 + h) + k <= 128 * (g ^ h) + i) for g in (2, 3)]
        masks[h] = np.concatenate(colsA + colsB, axis=1).astype(BF)  # [128,768]

    def pack_tiles(xTb):
        # [C, T] -> [128, tt*1024 + c*128 + t]
        return np.ascontiguousarray(
            xTb.reshape(NC_, 128, NTT, 128).transpose(1, 2, 0, 3)
            .reshape(128, NTT * 1024))

    in_maps = []
    for c in range(8):
        b, h = c % 4, c // 4
        xTb = xT[b] if h == 0 else xT1[b]
        in_maps.append({
            "xq": pack_tiles(xTb),
            "w": w_packed,
            "aux2": np.concatenate([masks[h], idn], axis=1),
        })

    res = bass_utils.run_bass_kernel_spmd(nc, in_maps, core_ids=list(range(8)),
                                          trace=_trace)
    _CACHE["last_results"] = res

    # Decode: U[c] [128, 8*260] -> [q_perm, 65]; un-permute h=1 tokens.
    O = np.empty((B, T, D), dtype=np.float32)
    for b in range(B):
        Uh = []
        for h in range(2):
            U = np.asarray(res.results[b + 4 * h]["out"],
                           dtype=np.float32)            # [128, 2080]
            U = U.reshape(128, NST, 4, 65).transpose(1, 2, 0, 3)
            U = U.reshape(T, 65)                          # permuted q order
            Uh.append(U[perm1] if h == 1 else U)          # global q order
        Ut = Uh[0] + Uh[1]
        O[b] = Ut[:, 0:64] / Ut[:, 64:65]
    return O
